# revision 1
# baseline (speedup 1.0000x reference)
"""GCN classifier (2x GCNConv + mean-pool + linear) on 8 Trainium2 NeuronCores.

Sharding: nodes (and their incident edges, partitioned by edge dst) are sharded
across the 8 cores; the small 128x128 weights are replicated; boundary node
features are exchanged with an AllGather of the scaled feature table after the
xw stage of each conv, before the per-edge gather/scatter.

All per-core differences are pushed into input *data* (the NEFF is SPMD: one
program for all 8 cores):
  - each core's edges are grouped into (dst-block of 128 nodes) x (src quadrant)
    cells, padded to a fixed number of 128-edge chunks (Cq) so the instruction
    stream is identical on every core
  - messages y[src] are fetched with dma_gather (int16 indices local to a src
    quadrant of 25000 rows), and scattered into PSUM with a one-hot(dst) matmul
  - degree counts / graph-id metadata are integer preprocessing done on host
"""

import math
import sys

sys.path.insert(0, "/opt/trn_rl_repo")

import ml_dtypes
import numpy as np

import concourse.bass as bass
import concourse.mybir as mybir
import concourse.tile as tile
from concourse import bacc
from concourse.bass_utils import run_bass_kernel_spmd
from concourse.masks import make_identity

BF16 = mybir.dt.bfloat16
F32 = mybir.dt.float32
I16 = mybir.dt.int16
I32 = mybir.dt.int32
NP_BF16 = ml_dtypes.bfloat16

P = 128
NCORES = 8

# problem sizes (hardcoded per the harness contract)
CFG = dict(N=100_000, E=1_600_000, G=1024, F=128, NCLS=10)

GB = 8  # dst blocks per gather group
EGB = 14  # node tiles per embedding-gather superchunk


def _plan(cfg):
    N, G = cfg["N"], cfg["G"]
    p = {}
    p["NPC"] = N // NCORES  # nodes per core
    p["NB"] = math.ceil(p["NPC"] / P)  # 128-node blocks per core
    p["NBP"] = p["NB"] * P
    p["QROWS"] = N // 4  # src quadrant rows (must be < 32768)
    assert p["QROWS"] < 32768
    p["groups"] = [
        list(range(g, min(g + GB, p["NB"]))) for g in range(0, p["NB"], GB)
    ]
    p["ESC"] = math.ceil(p["NB"] / EGB)  # embed superchunks
    p["ECOLS"] = EGB * P // 16  # idx cols per embed call
    p["NGT"] = G // P  # graph tiles
    assert G % P == 0
    return p


def _wrap16(idx_flat):
    """int16 index list -> [128, n/16] wrapped in 16 partitions, replicated 8x."""
    return np.tile(idx_flat.reshape(-1, 16).T, (8, 1))


def _prep_host(x, edge_index, batch, cfg):
    """Integer/index preprocessing + per-core metadata. Returns (per_core, Cq)."""
    pl = _plan(cfg)
    N, G = cfg["N"], cfg["G"]
    NPC, NB, NBP, QROWS = pl["NPC"], pl["NB"], pl["NBP"], pl["QROWS"]

    src = np.asarray(edge_index[0], np.int64)
    dst = np.asarray(edge_index[1], np.int64)
    batch = np.asarray(batch, np.int64)
    x = np.asarray(x, np.int64)

    deg_p1 = (np.bincount(dst, minlength=N) + 1).astype(np.float32)
    cnt = np.maximum(np.bincount(batch, minlength=G), 1).astype(np.float32)
    cnt_pt = cnt.reshape(pl["NGT"], P).T.copy()  # [P, NGT]

    core_of = dst // NPC
    per_core_edge = []  # (sorted sloc, sorted w, counts per cell)
    Cq = 1
    for k in range(NCORES):
        m = core_of == k
        s_k, d_k = src[m], dst[m] - k * NPC
        blk = d_k >> 7
        q = s_k // QROWS
        key = (blk * 4 + q).astype(np.int64)
        order = np.argsort(key, kind="stable")
        counts = np.bincount(key, minlength=NB * 4)
        Cq = max(Cq, math.ceil(counts.max() / P))
        sloc = (s_k - q * QROWS).astype(np.int16)[order]
        w = (d_k & 127).astype(np.float32)[order]
        per_core_edge.append((sloc, w, counts))

    per_core = []
    for k in range(NCORES):
        sloc, w, counts = per_core_edge[k]
        cap = Cq * P
        src_pad = np.zeros((NB * 4, cap), np.int16)
        dst_pad = np.full((NB * 4, cap), -1.0, np.float32)
        starts = np.concatenate([[0], np.cumsum(counts)])
        for cell in range(NB * 4):
            c0, c1 = starts[cell], starts[cell + 1]
            n = c1 - c0
            if n:
                src_pad[cell, :n] = sloc[c0:c1]
                dst_pad[cell, :n] = w[c0:c1]

        idx_cols, dst_cols = [], []
        for blocks in pl["groups"]:
            for q in range(4):
                cells = [b * 4 + q for b in blocks]
                flat = src_pad[cells].reshape(-1)
                idx_cols.append(_wrap16(flat))
            for b in blocks:
                # block-major: the 4*Cq chunk columns of block b, (q, cc) order
                cells = [b * 4 + q for q in range(4)]
                dst_cols.append(dst_pad[cells].reshape(-1, P).T)
        edge_idx = np.concatenate(idx_cols, 1)  # [128, TOTCOL] i16
        dstc = np.concatenate(dst_cols, 1).astype(NP_BF16)  # [128, NCH]

        # degree (layout [p, c] = local node c*128+p), pad nodes -> deg+1 = 1
        dp = np.ones(NBP, np.float32)
        dp[:NPC] = deg_p1[k * NPC : (k + 1) * NPC]
        dp = dp.reshape(NB, P).T.copy()

        # pool metadata
        bl = batch[k * NPC : (k + 1) * NPC]
        gbase = int(bl[0])
        gspan = int(bl[-1]) - gbase + 1
        assert gspan <= 2 * P, f"core {k} graph span {gspan} > 256"
        blf = np.full(NBP, -1.0, np.float32)
        blf[:NPC] = (bl - gbase).astype(np.float32)
        bl0 = blf.reshape(NB, P).T.astype(np.float32)
        bl1 = (blf - P).reshape(NB, P).T.astype(np.float32)
        gidx = np.zeros((P, 2), np.int32)
        for h in range(2):
            v = gbase + h * P + np.arange(P)
            v = np.where(v < G, v, G + (v % 8))
            gidx[:, h] = v

        # embedding gather indices (x values < 256 fit int16)
        xi = np.zeros((NBP, 3), np.int16)
        xi[:NPC] = x[k * NPC : (k + 1) * NPC].astype(np.int16)
        ecols = []
        for j in range(3):
            for s in range(pl["ESC"]):
                seg = np.zeros(EGB * P, np.int16)
                src_seg = xi[s * EGB * P : (s + 1) * EGB * P, j]
                seg[: len(src_seg)] = src_seg
                ecols.append(_wrap16(seg))
        emb_idx = np.concatenate(ecols, 1)

        per_core.append(
            dict(
                deg_p1=dp,
                bl0=bl0,
                bl1=bl1,
                gidx=gidx,
                cnt=cnt_pt,
                emb_idx=emb_idx,
                edge_idx=edge_idx,
                dst_cols=dstc,
            )
        )
    return per_core, Cq, pl


def _build(cfg, Cq, pl, totcol, nch, necol):
    """Build the SPMD Bass program (one NEFF for all 8 cores)."""
    import os
    PHASES = int(os.environ.get("K_PHASES", "9"))  # 1=embed 2=+conv1 3=+conv2 9=all
    SUB = int(os.environ.get("K_SUB", "9"))  # 1=xw+AG 2=+gathers 3=+onehot 4=+matmul/epi
    NOAG = int(os.environ.get("K_NOAG", "0"))  # 1: replace AllGather with local copies
    N, G, F, NCLS = cfg["N"], cfg["G"], cfg["F"], cfg["NCLS"]
    NPC, NB, QROWS, NGT = pl["NPC"], pl["NB"], pl["QROWS"], pl["NGT"]
    groups, ESC, ECOLS = pl["groups"], pl["ESC"], pl["ECOLS"]

    nc = bacc.Bacc("TRN2", num_devices=NCORES, num_swdge_queues=4)
    RG = [list(range(NCORES))]

    # ---- I/O ----
    tabs = [
        nc.dram_tensor("shape_tab", [16, F], F32, kind="ExternalInput"),
        nc.dram_tensor("color_tab", [16, F], F32, kind="ExternalInput"),
        nc.dram_tensor("pos_tab", [256, F], F32, kind="ExternalInput"),
    ]
    W1d = nc.dram_tensor("W1", [F, F], F32, kind="ExternalInput")
    W2d = nc.dram_tensor("W2", [F, F], F32, kind="ExternalInput")
    b1d = nc.dram_tensor("b1", [1, F], F32, kind="ExternalInput")
    b2d = nc.dram_tensor("b2", [1, F], F32, kind="ExternalInput")
    Wld = nc.dram_tensor("Wlin", [F, NCLS], F32, kind="ExternalInput")
    bld = nc.dram_tensor("blin", [1, NCLS], F32, kind="ExternalInput")
    degd = nc.dram_tensor("deg_p1", [P, NB], F32, kind="ExternalInput")
    bl0d = nc.dram_tensor("bl0", [P, NB], F32, kind="ExternalInput")
    bl1d = nc.dram_tensor("bl1", [P, NB], F32, kind="ExternalInput")
    gixd = nc.dram_tensor("gidx", [P, 2], I32, kind="ExternalInput")
    cntd = nc.dram_tensor("cnt", [P, NGT], F32, kind="ExternalInput")
    eixd = nc.dram_tensor("emb_idx", [P, necol], I16, kind="ExternalInput")
    xixd = nc.dram_tensor("edge_idx", [P, totcol], I16, kind="ExternalInput")
    dcd = nc.dram_tensor("dst_cols", [P, nch], BF16, kind="ExternalInput")
    outd = nc.dram_tensor("out", [G, NCLS], F32, kind="ExternalOutput")

    with tile.TileContext(nc) as tc:
        import contextlib

        ctx = contextlib.ExitStack()
        persist = ctx.enter_context(tc.tile_pool(name="persist", bufs=1))
        dramp = ctx.enter_context(tc.tile_pool(name="dramp", bufs=1, space="DRAM"))
        tp_pool = ctx.enter_context(tc.tile_pool(name="tp", bufs=2, space="PSUM"))
        xw_pool = ctx.enter_context(tc.tile_pool(name="xw", bufs=2, space="PSUM"))
        acc_pool = ctx.enter_context(tc.tile_pool(name="acc", bufs=2, space="PSUM"))
        pacc_pool = ctx.enter_context(tc.tile_pool(name="pacc", bufs=1, space="PSUM"))
        sb_pool = ctx.enter_context(tc.tile_pool(name="work", bufs=3))
        msg_pool = ctx.enter_context(tc.tile_pool(name="msg", bufs=5))
        oh_pool = ctx.enter_context(tc.tile_pool(name="oh", bufs=3))
        ix_pool = ctx.enter_context(tc.tile_pool(name="ix", bufs=4))
        craw = ctx.enter_context(tc.tile_pool(name="craw", bufs=1))

        def T(shape, dt, space=None, addr_space="Local", name=None):
            pool = dramp if space == "DRAM" else persist
            return pool.tile(shape, dt, tag=name, name=name, addr_space=addr_space)

        # ---- internal DRAM ----
        y_slice = [
            T([NPC, F], BF16, space="DRAM", name=f"y_slice{c}") for c in range(2)
        ]
        y_full = [
            T([NCORES * NPC, F], BF16, space="DRAM",
              addr_space="Local" if NOAG else "Shared", name=f"y_full{c}")
            for c in range(2)
        ]
        dram_sums = T([G + 8, F], F32, space="DRAM", name="dram_sums")
        ar_sums = T([G + 8, F], F32, space="DRAM", addr_space="Shared",
                          name="ar_sums")

        # ---- persistent SBUF ----
        hA = T([P, NB * F], BF16, name="hA")
        hB = T([P, NB * F], BF16, name="hB")
        y_nm = T([P, NB * F], BF16, name="y_nm")
        dstc_sb = T([P, nch], BF16, name="dstc_sb")
        nc.sync.dma_start(out=dstc_sb[:], in_=dcd[:])

        # constants
        iota_i = craw.tile([P, P], I32, tag="iota_i", name="iota_i")
        nc.gpsimd.iota(iota_i[:], pattern=[[1, P]], base=0, channel_multiplier=0)
        iota_bf = T([P, P], BF16, name="iota_bf")
        nc.vector.tensor_copy(iota_bf[:], iota_i[:])
        iota_f = T([P, P], F32, name="iota_f")
        nc.vector.tensor_copy(iota_f[:], iota_i[:])
        id_f32 = T([P, P], F32, name="id_f32")
        make_identity(nc, id_f32[:])
        id_bf = T([P, P], BF16, name="id_bf")
        nc.vector.tensor_copy(id_bf[:], id_f32[:])
        ones_row = T([1, P], F32, name="ones_row")
        nc.vector.memset(ones_row[:], 1.0)

        def load_cast(name, dram, shape, dt_in, dt_out):
            t = T(shape, dt_out, name=name)
            if dt_out == dt_in:
                nc.sync.dma_start(out=t[:], in_=dram[:])
            else:
                # NB: SWDGE cast-DMA + indirect_dma in one program crashes the
                # device (observed NRT_EXEC_UNIT_UNRECOVERABLE) - cast on DVE.
                raw = craw.tile(shape, dt_in, tag=name + "_r", name=name + "_r")
                nc.sync.dma_start(out=raw[:], in_=dram[:])
                nc.vector.tensor_copy(t[:], raw[:])
            return t

        Wc = [
            load_cast("W1", W1d, [F, F], F32, BF16),
            load_cast("W2", W2d, [F, F], F32, BF16),
        ]
        bc = [
            load_cast("b1", b1d, [1, F], F32, F32),
            load_cast("b2", b2d, [1, F], F32, F32),
        ]
        Wl_sb = load_cast("Wl", Wld, [F, NCLS], F32, F32)
        bl_sb = load_cast("bl", bld, [1, NCLS], F32, F32)
        bl0_sb = load_cast("bl0", bl0d, [P, NB], F32, F32)
        bl1_sb = load_cast("bl1", bl1d, [P, NB], F32, F32)
        cnt_sb = load_cast("cnt", cntd, [P, NGT], F32, F32)
        gix_sb = load_cast("gix", gixd, [P, 2], I32, I32)
        eix_sb = load_cast("eix", eixd, [P, necol], I16, I16)

        # dinv = 1/sqrt(deg+1); rdinv = sqrt(deg+1) (transposed for bias matmul)
        deg_sb = craw.tile([P, NB], F32, tag="deg_sb", name="deg_sb")
        nc.sync.dma_start(out=deg_sb[:], in_=degd[:])
        sq_sb = T([P, NB], F32, name="sq_sb")
        nc.scalar.sqrt(sq_sb[:], deg_sb[:])
        dinv = T([P, NB], F32, name="dinv")
        nc.vector.reciprocal(dinv[:], sq_sb[:])


        # zero dram_sums (pool scatter target) early
        zsb = craw.tile([P, 512], F32, tag="zsb", name="zsb")
        nc.vector.memset(zsb[:], 0.0)
        nrow = G + 8
        r = 0
        while r < nrow:
            take = min(512, ((nrow - r) // P) * P)
            pp = P
            if take == 0:
                take = nrow - r
                pp = take
            nc.sync.dma_start(
                out=dram_sums[r : r + take, :].rearrange("(c p) f -> p c f", p=pp),
                in_=zsb[:pp, : take * F // pp].rearrange("p (c f) -> p c f", f=F),
            )
            r += take

        # ---------------- embedding ----------------
        for s in range(ESC):
            t0 = s * EGB
            t1 = min(t0 + EGB, NB)
            nt = t1 - t0
            nidx = nt * P
            g_tiles = []
            for j in range(3):
                g = msg_pool.tile([P, EGB, F], F32, tag="msg")
                eoff = (j * ESC + s) * ECOLS
                nc.gpsimd.dma_gather(
                    out_ap=g[:, :nt, :],
                    in_ap=tabs[j][:, :],
                    idxs_ap=eix_sb[:, eoff : eoff + nidx // 16],
                    num_idxs=nidx,
                    num_idxs_reg=nidx,
                    elem_size=F,
                    single_packet=False,
                    queue_num=j,
                )
                g_tiles.append(g)
            nc.vector.tensor_tensor(
                out=g_tiles[0][:, :nt, :],
                in0=g_tiles[0][:, :nt, :],
                in1=g_tiles[1][:, :nt, :],
                op=mybir.AluOpType.add,
            )
            nc.vector.tensor_tensor(
                out=hA[:, t0 * F : t1 * F].rearrange("p (c f) -> p c f", f=F),
                in0=g_tiles[0][:, :nt, :],
                in1=g_tiles[2][:, :nt, :],
                op=mybir.AluOpType.add,
            )

        # ---------------- two GCN convs ----------------
        for conv in range(min(2, max(0, PHASES - 1))):
            hin = hA if conv == 0 else hB
            hout = hB if conv == 0 else hA

            # b_bcast[n, f] = b[f] replicated down partitions (rank-1 via PE)
            bb_ps = xw_pool.tile([P, P], F32, tag="xw")
            nc.tensor.matmul(bb_ps[:], lhsT=ones_row[:], rhs=bc[conv][:],
                             start=True, stop=True)
            b_bcast = craw.tile([P, P], F32, tag=f"b_bcast{conv}", name=f"b_bcast{conv}")
            nc.scalar.copy(b_bcast[:], bb_ps[:])

            # xw phase: y = dinv * (h @ W) -> y_nm (SBUF) and y_slice (DRAM)
            for t in range(NB):
                tp = tp_pool.tile([P, P], BF16, tag="tp")
                nc.tensor.transpose(tp[:], hin[:, t * F : (t + 1) * F], id_bf[:])
                hT = sb_pool.tile([P, P], BF16, tag="hT")
                nc.scalar.copy(hT[:], tp[:])
                xw = xw_pool.tile([P, P], F32, tag="xw")
                nc.tensor.matmul(xw[:], lhsT=hT[:], rhs=Wc[conv][:], start=True, stop=True)
                nc.scalar.activation(
                    y_nm[:, t * F : (t + 1) * F],
                    xw[:],
                    mybir.ActivationFunctionType.Copy,
                    scale=dinv[:, t : t + 1],
                )
            # y_nm -> y_slice DRAM (node-major rows)
            full_b = NPC // P
            nc.sync.dma_start(
                out=y_slice[conv][: full_b * P, :].rearrange("(c p) f -> p c f", p=P),
                in_=y_nm[:, : full_b * F].rearrange("p (c f) -> p c f", f=F),
            )
            rem = NPC - full_b * P
            if rem:
                nc.sync.dma_start(
                    out=y_slice[conv][full_b * P :, :],
                    in_=y_nm[:rem, full_b * F : (full_b + 1) * F],
                )
            if NOAG:
                for kk in range(NCORES):
                    nc.sync.dma_start(
                        out=y_full[conv][kk * NPC : (kk + 1) * NPC, :],
                        in_=y_slice[conv][:],
                    )
            else:
                nc.gpsimd.collective_compute(
                    "AllGather",
                    mybir.AluOpType.bypass,
                    replica_groups=RG,
                    ins=[y_slice[conv][:]],
                    outs=[y_full[conv][:]],
                )

            # scatter phase
            ch_off = 0  # chunk offset into dst_cols
            col_off = 0  # column offset into edge_idx
            for blocks in (groups if SUB >= 2 else []):
                nblk = len(blocks)
                nch_q = nblk * Cq
                nidx = nch_q * P
                msgs = []
                for q in range(4):
                    ixt = ix_pool.tile([P, GB * Cq * 8], I16, tag="ix")
                    nc.sync.dma_start(
                        out=ixt[:, : nidx // 16],
                        in_=xixd[:, col_off : col_off + nidx // 16],
                    )
                    col_off += nidx // 16
                    msg = msg_pool.tile([P, GB * Cq, F], BF16, tag="msg")
                    nc.gpsimd.dma_gather(
                        out_ap=msg[:, :nch_q, :],
                        in_ap=y_full[conv][q * QROWS : (q + 1) * QROWS, :],
                        idxs_ap=ixt[:, : nidx // 16],
                        num_idxs=nidx,
                        num_idxs_reg=nidx,
                        elem_size=F,
                        single_packet=False,
                        queue_num=q,
                    )
                    msgs.append(msg)
                if SUB < 3:
                    ch_off += 4 * nblk * Cq
                    continue
                for bi, b in enumerate(blocks):
                    oh = oh_pool.tile([P, 4 * Cq, P], BF16, tag="oh")
                    nc.vector.tensor_tensor(
                        out=oh[:],
                        in0=iota_bf[:].unsqueeze(1).broadcast_to([P, 4 * Cq, P]),
                        in1=dstc_sb[:, ch_off + bi * 4 * Cq : ch_off + (bi + 1) * 4 * Cq]
                        .unsqueeze(2)
                        .broadcast_to([P, 4 * Cq, P]),
                        op=mybir.AluOpType.is_equal,
                    )
                    if SUB < 4:
                        continue
                    acc = acc_pool.tile([P, P], F32, tag="acc")
                    j = 0
                    for q in range(4):
                        for cc in range(Cq):
                            nc.tensor.matmul(
                                acc[:],
                                lhsT=oh[:, q * Cq + cc, :],
                                rhs=msgs[q][:, bi * Cq + cc, :],
                                start=(j == 0),
                                stop=(j == 4 * Cq - 1),
                            )
                            j += 1
                    # bias: bb = b (x) rdinv (cancels the later *dinv); ACT op
                    bb = sb_pool.tile([P, P], F32, tag="bb")
                    nc.scalar.activation(
                        bb[:], b_bcast[:], mybir.ActivationFunctionType.Copy,
                        scale=sq_sb[:, b : b + 1],
                    )
                    hs = sb_pool.tile([P, P], F32, tag="ep")
                    nc.vector.tensor_tensor(
                        out=hs[:],
                        in0=acc[:],
                        in1=y_nm[:, b * F : (b + 1) * F],
                        op=mybir.AluOpType.add,
                    )
                    nc.vector.tensor_tensor(
                        out=hs[:], in0=hs[:], in1=bb[:], op=mybir.AluOpType.add,
                    )
                    nc.vector.tensor_tensor(
                        out=hs[:],
                        in0=hs[:],
                        in1=dinv[:, b : b + 1].to_broadcast([P, P]),
                        op=mybir.AluOpType.mult,
                    )
                    nc.scalar.activation(
                        hout[:, b * F : (b + 1) * F],
                        hs[:],
                        mybir.ActivationFunctionType.Relu,
                    )
                ch_off += 4 * nch_q

        # ---------------- global mean pool + linear ----------------
        do_pool = PHASES >= 9
        if do_pool:
            pacc = [pacc_pool.tile([P, P], F32, tag=f"pacc{h}", name=f"pacc{h}") for h in range(2)]
            bls = [bl0_sb, bl1_sb]
            for t in range(NB):
                for h in range(2):
                    oht = sb_pool.tile([P, P], BF16, tag="pooloh")
                    nc.vector.tensor_tensor(
                        out=oht[:],
                        in0=iota_f[:],
                        in1=bls[h][:, t : t + 1].to_broadcast([P, P]),
                        op=mybir.AluOpType.is_equal,
                    )
                    nc.tensor.matmul(
                        pacc[h][:],
                        lhsT=oht[:],
                        rhs=hA[:, t * F : (t + 1) * F],
                        start=(t == 0),
                        stop=(t == NB - 1),
                    )
            for h in range(2):
                se = sb_pool.tile([P, P], F32, tag="ep")
                nc.vector.tensor_copy(se[:], pacc[h][:])
                nc.gpsimd.indirect_dma_start(
                    out=dram_sums[:],
                    out_offset=bass.IndirectOffsetOnAxis(ap=gix_sb[:, h : h + 1], axis=0),
                    in_=se[:],
                    in_offset=None,
                )
            nc.gpsimd.collective_compute(
                "AllReduce",
                mybir.AluOpType.add,
                replica_groups=RG,
                ins=[dram_sums[:]],
                outs=[ar_sums[:]],
            )
            recip = T([P, NGT], F32, name="recip")
            nc.vector.reciprocal(recip[:], cnt_sb[:])
            for t in range(NGT):
                art = sb_pool.tile([P, P], F32, tag="art")
                nc.sync.dma_start(out=art[:], in_=ar_sums[t * P : (t + 1) * P, :])
                pooled = sb_pool.tile([P, P], F32, tag="ep")
                nc.vector.tensor_tensor(
                    out=pooled[:],
                    in0=art[:],
                    in1=recip[:, t : t + 1].to_broadcast([P, P]),
                    op=mybir.AluOpType.mult,
                )
                ptp = tp_pool.tile([P, P], F32, tag="tp")
                nc.tensor.transpose(ptp[:], pooled[:], id_f32[:])
                pooledT = sb_pool.tile([P, P], F32, tag="hT")
                nc.scalar.copy(pooledT[:], ptp[:])
                op = xw_pool.tile([P, NCLS], F32, tag="xw")
                nc.tensor.matmul(op[:], lhsT=pooledT[:], rhs=Wl_sb[:], start=True, stop=False)
                nc.tensor.matmul(op[:], lhsT=ones_row[:], rhs=bl_sb[:], start=False, stop=True)
                oute = sb_pool.tile([P, NCLS], F32, tag="oute")
                nc.vector.tensor_copy(oute[:], op[:])
                nc.sync.dma_start(out=outd[t * P : (t + 1) * P, :], in_=oute[:])


        else:
            dummy = sb_pool.tile([P, NCLS], F32, tag="oute", name="dummy")
            nc.vector.memset(dummy[:], 0.0)
            for t in range(NGT):
                nc.sync.dma_start(out=outd[t * P : (t + 1) * P, :], in_=dummy[:])

        ctx.close()
    nc.compile()
    return nc


_CACHE = {}


def _get_nc(cfg, Cq, pl, totcol, nch, necol):
    key = (tuple(sorted(cfg.items())), Cq, totcol, nch, necol)
    if key not in _CACHE:
        _CACHE[key] = _build(cfg, Cq, pl, totcol, nch, necol)
    return _CACHE[key]


def run(inputs, cfg, trace=False):
    x = np.asarray(inputs["x"])
    per_core, Cq, pl = _prep_host(x, np.asarray(inputs["edge_index"]),
                                  np.asarray(inputs["batch"]), cfg)
    totcol = per_core[0]["edge_idx"].shape[1]
    nch = per_core[0]["dst_cols"].shape[1]
    necol = per_core[0]["emb_idx"].shape[1]
    nc = _get_nc(cfg, Cq, pl, totcol, nch, necol)

    shared = dict(
        shape_tab=np.asarray(inputs["shape_tab"], np.float32),
        color_tab=np.asarray(inputs["color_tab"], np.float32),
        pos_tab=np.asarray(inputs["pos_tab"], np.float32),
        W1=np.asarray(inputs["W1"], np.float32),
        W2=np.asarray(inputs["W2"], np.float32),
        b1=np.asarray(inputs["b1"], np.float32).reshape(1, -1),
        b2=np.asarray(inputs["b2"], np.float32).reshape(1, -1),
        Wlin=np.asarray(inputs["Wlin"], np.float32),
        blin=np.asarray(inputs["blin"], np.float32).reshape(1, -1),
    )
    in_maps = [{**shared, **per_core[k]} for k in range(NCORES)]
    res = run_bass_kernel_spmd(nc, in_maps, list(range(NCORES)), trace=trace)
    out = np.asarray(res.results[0]["out"], np.float32)
    return out, res


def kernel(**inputs) -> np.ndarray:
    out, _ = run(inputs, CFG)
    return out



# revision 9
# speedup vs baseline: 1.3668x; 1.3668x over previous
"""GCN classifier (2x GCNConv + mean-pool + linear) on 8 Trainium2 NeuronCores.

Sharding: nodes (and their incident edges, partitioned by edge dst) are sharded
across the 8 cores; the small 128x128 weights are replicated; boundary node
features are exchanged with an AllGather of the scaled feature table after the
xw stage of each conv, before the per-edge gather/scatter.

All per-core differences are pushed into input *data* (the NEFF is SPMD: one
program for all 8 cores):
  - each core's edges are grouped into (dst-block of 128 nodes) x (src quadrant)
    cells, padded to a fixed number of 128-edge chunks (Cq) so the instruction
    stream is identical on every core
  - messages y[src] are fetched with dma_gather (int16 indices local to a src
    quadrant of 25000 rows), and scattered into PSUM with a one-hot(dst) matmul
  - degree counts / graph-id metadata are integer preprocessing done on host
"""

import math
import sys

sys.path.insert(0, "/opt/trn_rl_repo")

import ml_dtypes
import numpy as np

import concourse.bass as bass
import concourse.mybir as mybir
import concourse.tile as tile
from concourse import bacc
from concourse.bass_utils import run_bass_kernel_spmd
from concourse.masks import make_identity

BF16 = mybir.dt.bfloat16
F32 = mybir.dt.float32
I16 = mybir.dt.int16
I32 = mybir.dt.int32
NP_BF16 = ml_dtypes.bfloat16

P = 128
NCORES = 8

# problem sizes (hardcoded per the harness contract)
CFG = dict(N=100_000, E=1_600_000, G=1024, F=128, NCLS=10)

GB = 6  # dst blocks per gather group
EGB = 14  # node tiles per embedding-gather superchunk


def _plan(cfg):
    N, G = cfg["N"], cfg["G"]
    p = {}
    p["NPC"] = N // NCORES  # nodes per core
    p["NB"] = math.ceil(p["NPC"] / P)  # 128-node blocks per core
    p["NBP"] = p["NB"] * P
    p["QROWS"] = N // 4  # src quadrant rows (must be < 32768)
    assert p["QROWS"] < 32768
    p["groups"] = [
        list(range(g, min(g + GB, p["NB"]))) for g in range(0, p["NB"], GB)
    ]
    p["ESC"] = math.ceil(p["NB"] / EGB)  # embed superchunks
    p["ECOLS"] = EGB * P // 16  # idx cols per embed call
    p["NGT"] = G // P  # graph tiles
    assert G % P == 0
    return p


def _wrap16(idx_flat):
    """int16 index list -> [128, n/16] wrapped in 16 partitions, replicated 8x."""
    return np.tile(idx_flat.reshape(-1, 16).T, (8, 1))


def _prep_host(x, edge_index, batch, cfg):
    """Integer/index preprocessing + per-core metadata. Returns (per_core, Cq)."""
    pl = _plan(cfg)
    N, G = cfg["N"], cfg["G"]
    NPC, NB, NBP, QROWS = pl["NPC"], pl["NB"], pl["NBP"], pl["QROWS"]

    src = np.asarray(edge_index[0], np.int64)
    dst = np.asarray(edge_index[1], np.int64)
    batch = np.asarray(batch, np.int64)
    x = np.asarray(x, np.int64)

    deg_p1 = (np.bincount(dst, minlength=N) + 1).astype(np.float32)
    cnt = np.maximum(np.bincount(batch, minlength=G), 1).astype(np.float32)
    cnt_pt = cnt.reshape(pl["NGT"], P).T.copy()  # [P, NGT]

    core_of = dst // NPC
    per_core_edge = []  # (sorted sloc, sorted w, counts per cell)
    Cq = 1
    for k in range(NCORES):
        m = core_of == k
        s_k, d_k = src[m], dst[m] - k * NPC
        blk = d_k >> 7
        q = s_k // QROWS
        key = (blk * 4 + q).astype(np.int64)
        order = np.argsort(key, kind="stable")
        counts = np.bincount(key, minlength=NB * 4)
        Cq = max(Cq, math.ceil(counts.max() / P))
        sloc = (s_k - q * QROWS).astype(np.int16)[order]
        w = (d_k & 127).astype(np.float32)[order]
        per_core_edge.append((sloc, w, counts))

    per_core = []
    for k in range(NCORES):
        sloc, w, counts = per_core_edge[k]
        cap = Cq * P
        src_pad = np.zeros((NB * 4, cap), np.int16)
        dst_pad = np.full((NB * 4, cap), -1.0, np.float32)
        starts = np.concatenate([[0], np.cumsum(counts)])
        for cell in range(NB * 4):
            c0, c1 = starts[cell], starts[cell + 1]
            n = c1 - c0
            if n:
                src_pad[cell, :n] = sloc[c0:c1]
                dst_pad[cell, :n] = w[c0:c1]

        idx_cols, dst_cols = [], []
        for blocks in pl["groups"]:
            for q in range(4):
                cells = [b * 4 + q for b in blocks]
                flat = src_pad[cells].reshape(-1)
                idx_cols.append(_wrap16(flat))
            for b in blocks:
                # block-major: the 4*Cq chunk columns of block b, (q, cc) order
                cells = [b * 4 + q for q in range(4)]
                dst_cols.append(dst_pad[cells].reshape(-1, P).T)
        edge_idx = np.concatenate(idx_cols, 1)  # [128, TOTCOL] i16
        dstc = np.concatenate(dst_cols, 1).astype(NP_BF16)  # [128, NCH]

        # degree (layout [p, c] = local node c*128+p), pad nodes -> deg+1 = 1
        dp = np.ones(NBP, np.float32)
        dp[:NPC] = deg_p1[k * NPC : (k + 1) * NPC]
        dp = dp.reshape(NB, P).T.copy()

        # pool metadata
        bl = batch[k * NPC : (k + 1) * NPC]
        gbase = int(bl[0])
        gspan = int(bl[-1]) - gbase + 1
        assert gspan <= 2 * P, f"core {k} graph span {gspan} > 256"
        blf = np.full(NBP, -1.0, np.float32)
        blf[:NPC] = (bl - gbase).astype(np.float32)
        bl0 = blf.reshape(NB, P).T.astype(np.float32)
        bl1 = (blf - P).reshape(NB, P).T.astype(np.float32)
        gidx = np.zeros((P, 2), np.int32)
        for h in range(2):
            v = gbase + h * P + np.arange(P)
            v = np.where(v < G, v, G + (v % 8))
            gidx[:, h] = v

        # embedding gather indices (x values < 256 fit int16)
        xi = np.zeros((NBP, 3), np.int16)
        xi[:NPC] = x[k * NPC : (k + 1) * NPC].astype(np.int16)
        ecols = []
        for j in range(3):
            for s in range(pl["ESC"]):
                seg = np.zeros(EGB * P, np.int16)
                src_seg = xi[s * EGB * P : (s + 1) * EGB * P, j]
                seg[: len(src_seg)] = src_seg
                ecols.append(_wrap16(seg))
        emb_idx = np.concatenate(ecols, 1)

        per_core.append(
            dict(
                deg_p1=dp,
                bl0=bl0,
                bl1=bl1,
                gidx=gidx,
                cnt=cnt_pt,
                emb_idx=emb_idx,
                edge_idx=edge_idx,
                dst_cols=dstc,
            )
        )
    return per_core, Cq, pl


def _build(cfg, Cq, pl, totcol, nch, necol):
    """Build the SPMD Bass program (one NEFF for all 8 cores)."""
    import os
    PHASES = int(os.environ.get("K_PHASES", "9"))  # 1=embed 2=+conv1 3=+conv2 9=all
    SUB = int(os.environ.get("K_SUB", "9"))  # 1=xw+AG 2=+gathers 3=+onehot 4=+matmul/epi
    NOAG = int(os.environ.get("K_NOAG", "0"))  # 1: replace AllGather with local copies
    N, G, F, NCLS = cfg["N"], cfg["G"], cfg["F"], cfg["NCLS"]
    NPC, NB, QROWS, NGT = pl["NPC"], pl["NB"], pl["QROWS"], pl["NGT"]
    groups, ESC, ECOLS = pl["groups"], pl["ESC"], pl["ECOLS"]

    nc = bacc.Bacc("TRN2", num_devices=NCORES, num_swdge_queues=4)
    RG = [list(range(NCORES))]

    # ---- I/O ----
    tabs = [
        nc.dram_tensor("shape_tab", [16, F], BF16, kind="ExternalInput"),
        nc.dram_tensor("color_tab", [16, F], BF16, kind="ExternalInput"),
        nc.dram_tensor("pos_tab", [256, F], BF16, kind="ExternalInput"),
    ]
    W1d = nc.dram_tensor("W1", [F, F], F32, kind="ExternalInput")
    W2d = nc.dram_tensor("W2", [F, F], F32, kind="ExternalInput")
    b1d = nc.dram_tensor("b1", [1, F], F32, kind="ExternalInput")
    b2d = nc.dram_tensor("b2", [1, F], F32, kind="ExternalInput")
    Wld = nc.dram_tensor("Wlin", [F, NCLS], F32, kind="ExternalInput")
    bld = nc.dram_tensor("blin", [1, NCLS], F32, kind="ExternalInput")
    degd = nc.dram_tensor("deg_p1", [P, NB], F32, kind="ExternalInput")
    bl0d = nc.dram_tensor("bl0", [P, NB], F32, kind="ExternalInput")
    bl1d = nc.dram_tensor("bl1", [P, NB], F32, kind="ExternalInput")
    gixd = nc.dram_tensor("gidx", [P, 2], I32, kind="ExternalInput")
    cntd = nc.dram_tensor("cnt", [P, NGT], F32, kind="ExternalInput")
    eixd = nc.dram_tensor("emb_idx", [P, necol], I16, kind="ExternalInput")
    xixd = nc.dram_tensor("edge_idx", [P, totcol], I16, kind="ExternalInput")
    dcd = nc.dram_tensor("dst_cols", [P, nch], BF16, kind="ExternalInput")
    outd = nc.dram_tensor("out", [G, NCLS], F32, kind="ExternalOutput")

    with tile.TileContext(nc) as tc:
        import contextlib

        ctx = contextlib.ExitStack()
        persist = ctx.enter_context(tc.tile_pool(name="persist", bufs=1))
        dramp = ctx.enter_context(tc.tile_pool(name="dramp", bufs=1, space="DRAM"))
        tp_pool = ctx.enter_context(tc.tile_pool(name="tp", bufs=2, space="PSUM"))
        xw_pool = ctx.enter_context(tc.tile_pool(name="xw", bufs=2, space="PSUM"))
        acc_pool = ctx.enter_context(tc.tile_pool(name="acc", bufs=2, space="PSUM"))
        pacc_pool = ctx.enter_context(tc.tile_pool(name="pacc", bufs=1, space="PSUM"))
        sb_pool = ctx.enter_context(tc.tile_pool(name="work", bufs=3))
        msg_pool = ctx.enter_context(tc.tile_pool(name="msg", bufs=8))
        oh_pool = ctx.enter_context(tc.tile_pool(name="oh", bufs=3))
        ix_pool = ctx.enter_context(tc.tile_pool(name="ix", bufs=4))
        craw = ctx.enter_context(tc.tile_pool(name="craw", bufs=1))

        def T(shape, dt, space=None, addr_space="Local", name=None):
            pool = dramp if space == "DRAM" else persist
            return pool.tile(shape, dt, tag=name, name=name, addr_space=addr_space)

        # ---- internal DRAM ----
        y_slice = [
            T([NPC, F], BF16, space="DRAM", name=f"y_slice{c}") for c in range(2)
        ]
        y_full = [
            T([NCORES * NPC, F], BF16, space="DRAM",
              addr_space="Local" if NOAG else "Shared", name=f"y_full{c}")
            for c in range(2)
        ]
        dram_sums = T([G + 8, F], F32, space="DRAM", name="dram_sums")
        ar_sums = T([G + 8, F], F32, space="DRAM", addr_space="Shared",
                          name="ar_sums")

        # ---- persistent SBUF ----
        hA = T([P, NB * F], BF16, name="hA")
        hB = hA  # in-place: h(in) is dead once y_nm is computed in the xw phase
        y_nm = T([P, NB * F], BF16, name="y_nm")
        dstc_sb = T([P, nch], BF16, name="dstc_sb")
        nc.sync.dma_start(out=dstc_sb[:], in_=dcd[:])

        # constants
        iota_i = craw.tile([P, P], I32, tag="iota_i", name="iota_i")
        nc.gpsimd.iota(iota_i[:], pattern=[[1, P]], base=0, channel_multiplier=0)
        iota_bf = T([P, P], BF16, name="iota_bf")
        nc.vector.tensor_copy(iota_bf[:], iota_i[:])
        iota_f = T([P, P], F32, name="iota_f")
        nc.vector.tensor_copy(iota_f[:], iota_i[:])
        id_f32 = T([P, P], F32, name="id_f32")
        make_identity(nc, id_f32[:])
        id_bf = T([P, P], BF16, name="id_bf")
        nc.vector.tensor_copy(id_bf[:], id_f32[:])
        ones_row = T([1, P], F32, name="ones_row")
        nc.vector.memset(ones_row[:], 1.0)

        def load_cast(name, dram, shape, dt_in, dt_out):
            t = T(shape, dt_out, name=name)
            if dt_out == dt_in:
                nc.sync.dma_start(out=t[:], in_=dram[:])
            else:
                # NB: SWDGE cast-DMA + indirect_dma in one program crashes the
                # device (observed NRT_EXEC_UNIT_UNRECOVERABLE) - cast on DVE.
                raw = craw.tile(shape, dt_in, tag=name + "_r", name=name + "_r")
                nc.sync.dma_start(out=raw[:], in_=dram[:])
                nc.vector.tensor_copy(t[:], raw[:])
            return t

        Wc = [
            load_cast("W1", W1d, [F, F], F32, BF16),
            load_cast("W2", W2d, [F, F], F32, BF16),
        ]
        bc = [
            load_cast("b1", b1d, [1, F], F32, F32),
            load_cast("b2", b2d, [1, F], F32, F32),
        ]
        Wl_sb = load_cast("Wl", Wld, [F, NCLS], F32, F32)
        bl_sb = load_cast("bl", bld, [1, NCLS], F32, F32)
        bl0_sb = load_cast("bl0", bl0d, [P, NB], F32, F32)
        bl1_sb = load_cast("bl1", bl1d, [P, NB], F32, F32)
        cnt_sb = load_cast("cnt", cntd, [P, NGT], F32, F32)
        gix_sb = load_cast("gix", gixd, [P, 2], I32, I32)
        eix_sb = load_cast("eix", eixd, [P, necol], I16, I16)

        # dinv = 1/sqrt(deg+1); rdinv = sqrt(deg+1) (transposed for bias matmul)
        deg_sb = craw.tile([P, NB], F32, tag="deg_sb", name="deg_sb")
        nc.sync.dma_start(out=deg_sb[:], in_=degd[:])
        sq_sb = T([P, NB], F32, name="sq_sb")
        nc.scalar.sqrt(sq_sb[:], deg_sb[:])
        dinv = T([P, NB], F32, name="dinv")
        nc.vector.reciprocal(dinv[:], sq_sb[:])


        # zero dram_sums (pool scatter target) early
        zsb = craw.tile([P, 512], F32, tag="zsb", name="zsb")
        nc.vector.memset(zsb[:], 0.0)
        nrow = G + 8
        r = 0
        while r < nrow:
            take = min(512, ((nrow - r) // P) * P)
            pp = P
            if take == 0:
                take = nrow - r
                pp = take
            nc.sync.dma_start(
                out=dram_sums[r : r + take, :].rearrange("(c p) f -> p c f", p=pp),
                in_=zsb[:pp, : take * F // pp].rearrange("p (c f) -> p c f", f=F),
            )
            r += take

        # ---------------- embedding ----------------
        for s in range(ESC):
            t0 = s * EGB
            t1 = min(t0 + EGB, NB)
            nt = t1 - t0
            nidx = nt * P
            g_tiles = []
            for j in range(3):
                g = msg_pool.tile([P, EGB, F], BF16, tag="msg")
                eoff = (j * ESC + s) * ECOLS
                nc.gpsimd.dma_gather(
                    out_ap=g[:, :nt, :],
                    in_ap=tabs[j][:, :],
                    idxs_ap=eix_sb[:, eoff : eoff + nidx // 16],
                    num_idxs=nidx,
                    num_idxs_reg=nidx,
                    elem_size=F,
                    single_packet=False,
                    queue_num=j,
                )
                g_tiles.append(g)
            nc.vector.tensor_tensor(
                out=g_tiles[0][:, :nt, :],
                in0=g_tiles[0][:, :nt, :],
                in1=g_tiles[1][:, :nt, :],
                op=mybir.AluOpType.add,
            )
            nc.vector.tensor_tensor(
                out=hA[:, t0 * F : t1 * F].rearrange("p (c f) -> p c f", f=F),
                in0=g_tiles[0][:, :nt, :],
                in1=g_tiles[2][:, :nt, :],
                op=mybir.AluOpType.add,
            )

        # ---------------- two GCN convs ----------------
        do_pool = PHASES >= 9
        pacc = [
            pacc_pool.tile([P, P], F32, tag=f"pacc{h}", name=f"pacc{h}")
            for h in range(2)
        ]
        for conv in range(min(2, max(0, PHASES - 1))):
            hin = hA if conv == 0 else hB
            hout = hB if conv == 0 else hA

            # b_bcast[n, f] = b[f] replicated down partitions (rank-1 via PE)
            bb_ps = xw_pool.tile([P, P], F32, tag="xw")
            nc.tensor.matmul(bb_ps[:], lhsT=ones_row[:], rhs=bc[conv][:],
                             start=True, stop=True)
            b_bcast = craw.tile([P, P], F32, tag=f"b_bcast{conv}", name=f"b_bcast{conv}")
            nc.scalar.copy(b_bcast[:], bb_ps[:])

            # xw phase: y = dinv * (h @ W) -> y_nm (SBUF) and y_slice (DRAM)
            for t in range(NB):
                tp = tp_pool.tile([P, P], BF16, tag="tp")
                nc.tensor.transpose(tp[:], hin[:, t * F : (t + 1) * F], id_bf[:])
                hT = sb_pool.tile([P, P], BF16, tag="hT")
                nc.scalar.copy(hT[:], tp[:])
                xw = xw_pool.tile([P, P], F32, tag="xw")
                nc.tensor.matmul(xw[:], lhsT=hT[:], rhs=Wc[conv][:], start=True, stop=True)
                nc.scalar.activation(
                    y_nm[:, t * F : (t + 1) * F],
                    xw[:],
                    mybir.ActivationFunctionType.Copy,
                    scale=dinv[:, t : t + 1],
                )
            # y_nm -> y_slice DRAM (node-major rows)
            full_b = NPC // P
            nc.sync.dma_start(
                out=y_slice[conv][: full_b * P, :].rearrange("(c p) f -> p c f", p=P),
                in_=y_nm[:, : full_b * F].rearrange("p (c f) -> p c f", f=F),
            )
            rem = NPC - full_b * P
            if rem:
                nc.sync.dma_start(
                    out=y_slice[conv][full_b * P :, :],
                    in_=y_nm[:rem, full_b * F : (full_b + 1) * F],
                )
            if NOAG:
                for kk in range(NCORES):
                    nc.sync.dma_start(
                        out=y_full[conv][kk * NPC : (kk + 1) * NPC, :],
                        in_=y_slice[conv][:],
                    )
            else:
                nc.gpsimd.collective_compute(
                    "AllGather",
                    mybir.AluOpType.bypass,
                    replica_groups=RG,
                    ins=[y_slice[conv][:]],
                    outs=[y_full[conv][:]],
                )

            # scatter phase
            ch_off = 0  # chunk offset into dst_cols
            col_off = 0  # column offset into edge_idx
            for blocks in (groups if SUB >= 2 else []):
                nblk = len(blocks)
                nch_q = nblk * Cq
                nidx = nch_q * P
                msgs = []
                for q in range(4):
                    ixt = ix_pool.tile([P, GB * Cq * 8], I16, tag="ix")
                    nc.sync.dma_start(
                        out=ixt[:, : nidx // 16],
                        in_=xixd[:, col_off : col_off + nidx // 16],
                    )
                    col_off += nidx // 16
                    msg = msg_pool.tile([P, GB * Cq, F], BF16, tag="msg")
                    nc.gpsimd.dma_gather(
                        out_ap=msg[:, :nch_q, :],
                        in_ap=y_full[conv][q * QROWS : (q + 1) * QROWS, :],
                        idxs_ap=ixt[:, : nidx // 16],
                        num_idxs=nidx,
                        num_idxs_reg=nidx,
                        elem_size=F,
                        single_packet=False,
                        queue_num=q,
                    )
                    msgs.append(msg)
                if SUB < 3:
                    ch_off += 4 * nblk * Cq
                    continue
                for bi, b in enumerate(blocks):
                    oh = oh_pool.tile([P, 4 * Cq, P], BF16, tag="oh")
                    nc.vector.tensor_tensor(
                        out=oh[:],
                        in0=iota_bf[:].unsqueeze(1).broadcast_to([P, 4 * Cq, P]),
                        in1=dstc_sb[:, ch_off + bi * 4 * Cq : ch_off + (bi + 1) * 4 * Cq]
                        .unsqueeze(2)
                        .broadcast_to([P, 4 * Cq, P]),
                        op=mybir.AluOpType.is_equal,
                    )
                    if SUB < 4:
                        continue
                    acc = acc_pool.tile([P, P], F32, tag="acc")
                    j = 0
                    for q in range(4):
                        for cc in range(Cq):
                            nc.tensor.matmul(
                                acc[:],
                                lhsT=oh[:, q * Cq + cc, :],
                                rhs=msgs[q][:, bi * Cq + cc, :],
                                start=(j == 0),
                                stop=(j == 4 * Cq - 1),
                            )
                            j += 1
                    # bias: bb = b (x) rdinv (cancels the later *dinv); ACT op
                    bb = sb_pool.tile([P, P], F32, tag="bb")
                    nc.scalar.activation(
                        bb[:], b_bcast[:], mybir.ActivationFunctionType.Copy,
                        scale=sq_sb[:, b : b + 1],
                    )
                    hs = sb_pool.tile([P, P], F32, tag="ep")
                    nc.vector.tensor_tensor(
                        out=hs[:],
                        in0=acc[:],
                        in1=y_nm[:, b * F : (b + 1) * F],
                        op=mybir.AluOpType.add,
                    )
                    nc.vector.tensor_tensor(
                        out=hs[:], in0=hs[:], in1=bb[:], op=mybir.AluOpType.add,
                    )
                    nc.vector.tensor_tensor(
                        out=hs[:],
                        in0=hs[:],
                        in1=dinv[:, b : b + 1].to_broadcast([P, P]),
                        op=mybir.AluOpType.mult,
                    )
                    nc.scalar.activation(
                        hout[:, b * F : (b + 1) * F],
                        hs[:],
                        mybir.ActivationFunctionType.Relu,
                    )
                    # mean-pool accumulation interleaved into conv2's epilogue
                    if conv == 1 and do_pool:
                        for hh in range(2):
                            bls = bl0_sb if hh == 0 else bl1_sb
                            oht = sb_pool.tile([P, P], BF16, tag="pooloh")
                            nc.vector.tensor_tensor(
                                out=oht[:],
                                in0=iota_f[:],
                                in1=bls[:, b : b + 1].to_broadcast([P, P]),
                                op=mybir.AluOpType.is_equal,
                            )
                            nc.tensor.matmul(
                                pacc[hh][:],
                                lhsT=oht[:],
                                rhs=hA[:, b * F : (b + 1) * F],
                                start=(b == 0),
                                stop=(b == NB - 1),
                            )
                ch_off += 4 * nch_q

        # ---------------- global mean pool + linear ----------------
        if do_pool:
            for h in range(2):
                se = sb_pool.tile([P, P], F32, tag="ep")
                nc.vector.tensor_copy(se[:], pacc[h][:])
                nc.gpsimd.indirect_dma_start(
                    out=dram_sums[:],
                    out_offset=bass.IndirectOffsetOnAxis(ap=gix_sb[:, h : h + 1], axis=0),
                    in_=se[:],
                    in_offset=None,
                )
            nc.gpsimd.collective_compute(
                "AllReduce",
                mybir.AluOpType.add,
                replica_groups=RG,
                ins=[dram_sums[:]],
                outs=[ar_sums[:]],
            )
            recip = T([P, NGT], F32, name="recip")
            nc.vector.reciprocal(recip[:], cnt_sb[:])
            for t in range(NGT):
                art = sb_pool.tile([P, P], F32, tag="art")
                nc.sync.dma_start(out=art[:], in_=ar_sums[t * P : (t + 1) * P, :])
                pooled = sb_pool.tile([P, P], F32, tag="ep")
                nc.vector.tensor_tensor(
                    out=pooled[:],
                    in0=art[:],
                    in1=recip[:, t : t + 1].to_broadcast([P, P]),
                    op=mybir.AluOpType.mult,
                )
                ptp = tp_pool.tile([P, P], F32, tag="tp")
                nc.tensor.transpose(ptp[:], pooled[:], id_f32[:])
                pooledT = sb_pool.tile([P, P], F32, tag="hT")
                nc.scalar.copy(pooledT[:], ptp[:])
                op = xw_pool.tile([P, NCLS], F32, tag="xw")
                nc.tensor.matmul(op[:], lhsT=pooledT[:], rhs=Wl_sb[:], start=True, stop=False)
                nc.tensor.matmul(op[:], lhsT=ones_row[:], rhs=bl_sb[:], start=False, stop=True)
                oute = sb_pool.tile([P, NCLS], F32, tag="oute")
                nc.vector.tensor_copy(oute[:], op[:])
                nc.sync.dma_start(out=outd[t * P : (t + 1) * P, :], in_=oute[:])


        else:
            dummy = sb_pool.tile([P, NCLS], F32, tag="oute", name="dummy")
            nc.vector.memset(dummy[:], 0.0)
            for t in range(NGT):
                nc.sync.dma_start(out=outd[t * P : (t + 1) * P, :], in_=dummy[:])

        ctx.close()
    nc.compile()
    return nc


_CACHE = {}


def _get_nc(cfg, Cq, pl, totcol, nch, necol):
    key = (tuple(sorted(cfg.items())), Cq, totcol, nch, necol)
    if key not in _CACHE:
        _CACHE[key] = _build(cfg, Cq, pl, totcol, nch, necol)
    return _CACHE[key]


def run(inputs, cfg, trace=False):
    x = np.asarray(inputs["x"])
    per_core, Cq, pl = _prep_host(x, np.asarray(inputs["edge_index"]),
                                  np.asarray(inputs["batch"]), cfg)
    totcol = per_core[0]["edge_idx"].shape[1]
    nch = per_core[0]["dst_cols"].shape[1]
    necol = per_core[0]["emb_idx"].shape[1]
    nc = _get_nc(cfg, Cq, pl, totcol, nch, necol)

    shared = dict(
        shape_tab=np.asarray(inputs["shape_tab"], np.float32).astype(NP_BF16),
        color_tab=np.asarray(inputs["color_tab"], np.float32).astype(NP_BF16),
        pos_tab=np.asarray(inputs["pos_tab"], np.float32).astype(NP_BF16),
        W1=np.asarray(inputs["W1"], np.float32),
        W2=np.asarray(inputs["W2"], np.float32),
        b1=np.asarray(inputs["b1"], np.float32).reshape(1, -1),
        b2=np.asarray(inputs["b2"], np.float32).reshape(1, -1),
        Wlin=np.asarray(inputs["Wlin"], np.float32),
        blin=np.asarray(inputs["blin"], np.float32).reshape(1, -1),
    )
    in_maps = [{**shared, **per_core[k]} for k in range(NCORES)]
    res = run_bass_kernel_spmd(nc, in_maps, list(range(NCORES)), trace=trace)
    out = np.asarray(res.results[0]["out"], np.float32)
    return out, res


def kernel(**inputs) -> np.ndarray:
    out, _ = run(inputs, CFG)
    return out



# revision 10
# speedup vs baseline: 1.5930x; 1.1655x over previous
"""GCN classifier (2x GCNConv + mean-pool + linear) on 8 Trainium2 NeuronCores.

Sharding: nodes (and their incident edges, partitioned by edge dst) are sharded
across the 8 cores; the small 128x128 weights are replicated; boundary node
features are exchanged with an AllGather of the scaled feature table after the
xw stage of each conv, before the per-edge gather/scatter.

All per-core differences are pushed into input *data* (the NEFF is SPMD: one
program for all 8 cores):
  - each core's edges are grouped into (dst-block of 128 nodes) x (src quadrant)
    cells, padded to a fixed number of 128-edge chunks (Cq) so the instruction
    stream is identical on every core
  - messages y[src] are fetched with dma_gather (int16 indices local to a src
    quadrant of 25000 rows), and scattered into PSUM with a one-hot(dst) matmul
  - degree counts / graph-id metadata are integer preprocessing done on host
"""

import math
import sys

sys.path.insert(0, "/opt/trn_rl_repo")

import ml_dtypes
import numpy as np

import concourse.bass as bass
import concourse.mybir as mybir
import concourse.tile as tile
from concourse import bacc
from concourse.bass_utils import run_bass_kernel_spmd
from concourse.masks import make_identity

BF16 = mybir.dt.bfloat16
F32 = mybir.dt.float32
I16 = mybir.dt.int16
I32 = mybir.dt.int32
NP_BF16 = ml_dtypes.bfloat16

P = 128
NCORES = 8

# problem sizes (hardcoded per the harness contract)
CFG = dict(N=100_000, E=1_600_000, G=1024, F=128, NCLS=10)

GB = 6  # dst blocks per gather group
EGB = 14  # node tiles per embedding-gather superchunk


def _plan(cfg):
    N, G = cfg["N"], cfg["G"]
    p = {}
    p["NPC"] = N // NCORES  # nodes per core
    p["NB"] = math.ceil(p["NPC"] / P)  # 128-node blocks per core
    p["NBP"] = p["NB"] * P
    p["QROWS"] = N // 4  # src quadrant rows (must be < 32768)
    assert p["QROWS"] < 32768
    p["groups"] = [
        list(range(g, min(g + GB, p["NB"]))) for g in range(0, p["NB"], GB)
    ]
    p["ESC"] = math.ceil(p["NB"] / EGB)  # embed superchunks
    p["ECOLS"] = EGB * P // 16  # idx cols per embed call
    p["NGT"] = G // P  # graph tiles
    assert G % P == 0
    return p


def _wrap16(idx_flat):
    """int16 index list -> [128, n/16] wrapped in 16 partitions, replicated 8x."""
    return np.tile(idx_flat.reshape(-1, 16).T, (8, 1))


def _prep_host(x, edge_index, batch, cfg):
    """Integer/index preprocessing + per-core metadata. Returns (per_core, Cq)."""
    pl = _plan(cfg)
    N, G = cfg["N"], cfg["G"]
    NPC, NB, NBP, QROWS = pl["NPC"], pl["NB"], pl["NBP"], pl["QROWS"]

    src = np.asarray(edge_index[0], np.int64)
    dst = np.asarray(edge_index[1], np.int64)
    batch = np.asarray(batch, np.int64)
    x = np.asarray(x, np.int64)

    deg_p1 = (np.bincount(dst, minlength=N) + 1).astype(np.float32)
    cnt = np.maximum(np.bincount(batch, minlength=G), 1).astype(np.float32)
    cnt_pt = cnt.reshape(pl["NGT"], P).T.copy()  # [P, NGT]

    core_of = dst // NPC
    per_core_edge = []  # (sorted sloc, sorted w, counts per cell)
    Cq = 1
    for k in range(NCORES):
        m = core_of == k
        s_k, d_k = src[m], dst[m] - k * NPC
        blk = d_k >> 7
        q = s_k // QROWS
        key = (blk * 4 + q).astype(np.int64)
        order = np.argsort(key, kind="stable")
        counts = np.bincount(key, minlength=NB * 4)
        Cq = max(Cq, math.ceil(counts.max() / P))
        sloc = (s_k - q * QROWS).astype(np.int16)[order]
        w = (d_k & 127).astype(np.float32)[order]
        per_core_edge.append((sloc, w, counts))

    per_core = []
    for k in range(NCORES):
        sloc, w, counts = per_core_edge[k]
        cap = Cq * P
        src_pad = np.zeros((NB * 4, cap), np.int16)
        dst_pad = np.full((NB * 4, cap), -1.0, np.float32)
        starts = np.concatenate([[0], np.cumsum(counts)])
        for cell in range(NB * 4):
            c0, c1 = starts[cell], starts[cell + 1]
            n = c1 - c0
            if n:
                src_pad[cell, :n] = sloc[c0:c1]
                dst_pad[cell, :n] = w[c0:c1]

        idx_cols, dst_cols = [], []
        for blocks in pl["groups"]:
            for q in range(4):
                cells = [b * 4 + q for b in blocks]
                flat = src_pad[cells].reshape(-1)
                idx_cols.append(_wrap16(flat))
            for b in blocks:
                # block-major: the 4*Cq chunk columns of block b, (q, cc) order
                cells = [b * 4 + q for q in range(4)]
                dst_cols.append(dst_pad[cells].reshape(-1, P).T)
        edge_idx = np.concatenate(idx_cols, 1)  # [128, TOTCOL] i16
        dstc = np.concatenate(dst_cols, 1).astype(NP_BF16)  # [128, NCH]

        # degree (layout [p, c] = local node c*128+p), pad nodes -> deg+1 = 1
        dp = np.ones(NBP, np.float32)
        dp[:NPC] = deg_p1[k * NPC : (k + 1) * NPC]
        dp = dp.reshape(NB, P).T.copy()

        # pool metadata
        bl = batch[k * NPC : (k + 1) * NPC]
        gbase = int(bl[0])
        gspan = int(bl[-1]) - gbase + 1
        assert gspan <= 2 * P, f"core {k} graph span {gspan} > 256"
        blf = np.full(NBP, -1.0, np.float32)
        blf[:NPC] = (bl - gbase).astype(np.float32)
        bl0 = blf.reshape(NB, P).T.astype(np.float32)
        bl1 = (blf - P).reshape(NB, P).T.astype(np.float32)
        gidx = np.zeros((P, 2), np.int32)
        for h in range(2):
            v = gbase + h * P + np.arange(P)
            v = np.where(v < G, v, G + (v % 8))
            gidx[:, h] = v

        # embedding gather indices (x values < 256 fit int16)
        xi = np.zeros((NBP, 3), np.int16)
        xi[:NPC] = x[k * NPC : (k + 1) * NPC].astype(np.int16)
        ecols = []
        for j in range(3):
            for s in range(pl["ESC"]):
                seg = np.zeros(EGB * P, np.int16)
                src_seg = xi[s * EGB * P : (s + 1) * EGB * P, j]
                seg[: len(src_seg)] = src_seg
                ecols.append(_wrap16(seg))
        emb_idx = np.concatenate(ecols, 1)

        per_core.append(
            dict(
                deg_p1=dp,
                bl0=bl0,
                bl1=bl1,
                gidx=gidx,
                cnt=cnt_pt,
                emb_idx=emb_idx,
                edge_idx=edge_idx,
                dst_cols=dstc,
            )
        )
    return per_core, Cq, pl


def _build(cfg, Cq, pl, totcol, nch, necol):
    """Build the SPMD Bass program (one NEFF for all 8 cores)."""
    import os
    PHASES = int(os.environ.get("K_PHASES", "9"))  # 1=embed 2=+conv1 3=+conv2 9=all
    SUB = int(os.environ.get("K_SUB", "9"))  # 1=xw+AG 2=+gathers 3=+onehot 4=+matmul/epi
    NOAG = int(os.environ.get("K_NOAG", "0"))  # 1: replace AllGather with local copies
    N, G, F, NCLS = cfg["N"], cfg["G"], cfg["F"], cfg["NCLS"]
    NPC, NB, QROWS, NGT = pl["NPC"], pl["NB"], pl["QROWS"], pl["NGT"]
    groups, ESC, ECOLS = pl["groups"], pl["ESC"], pl["ECOLS"]

    nc = bacc.Bacc("TRN2", num_devices=NCORES, num_swdge_queues=4)
    RG = [list(range(NCORES))]

    # ---- I/O ----
    tabs = [
        nc.dram_tensor("shape_tab", [16, F], BF16, kind="ExternalInput"),
        nc.dram_tensor("color_tab", [16, F], BF16, kind="ExternalInput"),
        nc.dram_tensor("pos_tab", [256, F], BF16, kind="ExternalInput"),
    ]
    W1d = nc.dram_tensor("W1", [F, F], F32, kind="ExternalInput")
    W2d = nc.dram_tensor("W2", [F, F], F32, kind="ExternalInput")
    b1d = nc.dram_tensor("b1", [1, F], F32, kind="ExternalInput")
    b2d = nc.dram_tensor("b2", [1, F], F32, kind="ExternalInput")
    Wld = nc.dram_tensor("Wlin", [F, NCLS], F32, kind="ExternalInput")
    bld = nc.dram_tensor("blin", [1, NCLS], F32, kind="ExternalInput")
    degd = nc.dram_tensor("deg_p1", [P, NB], F32, kind="ExternalInput")
    bl0d = nc.dram_tensor("bl0", [P, NB], F32, kind="ExternalInput")
    bl1d = nc.dram_tensor("bl1", [P, NB], F32, kind="ExternalInput")
    gixd = nc.dram_tensor("gidx", [P, 2], I32, kind="ExternalInput")
    cntd = nc.dram_tensor("cnt", [P, NGT], F32, kind="ExternalInput")
    eixd = nc.dram_tensor("emb_idx", [P, necol], I16, kind="ExternalInput")
    xixd = nc.dram_tensor("edge_idx", [P, totcol], I16, kind="ExternalInput")
    dcd = nc.dram_tensor("dst_cols", [P, nch], BF16, kind="ExternalInput")
    outd = nc.dram_tensor("out", [G, NCLS], F32, kind="ExternalOutput")

    with tile.TileContext(nc) as tc:
        import contextlib

        ctx = contextlib.ExitStack()
        persist = ctx.enter_context(tc.tile_pool(name="persist", bufs=1))
        dramp = ctx.enter_context(tc.tile_pool(name="dramp", bufs=1, space="DRAM"))
        tp_pool = ctx.enter_context(tc.tile_pool(name="tp", bufs=2, space="PSUM"))
        xw_pool = ctx.enter_context(tc.tile_pool(name="xw", bufs=2, space="PSUM"))
        acc_pool = ctx.enter_context(tc.tile_pool(name="acc", bufs=2, space="PSUM"))
        pacc_pool = ctx.enter_context(tc.tile_pool(name="pacc", bufs=1, space="PSUM"))
        sb_pool = ctx.enter_context(tc.tile_pool(name="work", bufs=3))
        msg_pool = ctx.enter_context(tc.tile_pool(name="msg", bufs=8))
        oh_pool = ctx.enter_context(tc.tile_pool(name="oh", bufs=3))
        ix_pool = ctx.enter_context(tc.tile_pool(name="ix", bufs=8))
        craw = ctx.enter_context(tc.tile_pool(name="craw", bufs=1))

        def T(shape, dt, space=None, addr_space="Local", name=None):
            pool = dramp if space == "DRAM" else persist
            return pool.tile(shape, dt, tag=name, name=name, addr_space=addr_space)

        # ---- internal DRAM ----
        y_slice = [
            T([NPC, F], BF16, space="DRAM", name=f"y_slice{c}") for c in range(2)
        ]
        y_full = [
            T([NCORES * NPC, F], BF16, space="DRAM",
              addr_space="Local" if NOAG else "Shared", name=f"y_full{c}")
            for c in range(2)
        ]
        dram_sums = T([G + 8, F], F32, space="DRAM", name="dram_sums")
        ar_sums = T([G + 8, F], F32, space="DRAM", addr_space="Shared",
                          name="ar_sums")

        # ---- persistent SBUF ----
        hA = T([P, NB * F], BF16, name="hA")
        hB = hA  # in-place: h(in) is dead once y_nm is computed in the xw phase
        y_nm = T([P, NB * F], BF16, name="y_nm")
        dstc_sb = T([P, nch], BF16, name="dstc_sb")
        nc.sync.dma_start(out=dstc_sb[:], in_=dcd[:])

        # constants
        iota_i = craw.tile([P, P], I32, tag="iota_i", name="iota_i")
        nc.gpsimd.iota(iota_i[:], pattern=[[1, P]], base=0, channel_multiplier=0)
        iota_bf = T([P, P], BF16, name="iota_bf")
        nc.vector.tensor_copy(iota_bf[:], iota_i[:])
        iota_f = T([P, P], F32, name="iota_f")
        nc.vector.tensor_copy(iota_f[:], iota_i[:])
        id_f32 = T([P, P], F32, name="id_f32")
        make_identity(nc, id_f32[:])
        id_bf = T([P, P], BF16, name="id_bf")
        nc.vector.tensor_copy(id_bf[:], id_f32[:])
        ones_row = T([1, P], F32, name="ones_row")
        nc.vector.memset(ones_row[:], 1.0)

        def load_cast(name, dram, shape, dt_in, dt_out):
            t = T(shape, dt_out, name=name)
            if dt_out == dt_in:
                nc.sync.dma_start(out=t[:], in_=dram[:])
            else:
                # NB: SWDGE cast-DMA + indirect_dma in one program crashes the
                # device (observed NRT_EXEC_UNIT_UNRECOVERABLE) - cast on DVE.
                raw = craw.tile(shape, dt_in, tag=name + "_r", name=name + "_r")
                nc.sync.dma_start(out=raw[:], in_=dram[:])
                nc.vector.tensor_copy(t[:], raw[:])
            return t

        Wc = [
            load_cast("W1", W1d, [F, F], F32, BF16),
            load_cast("W2", W2d, [F, F], F32, BF16),
        ]
        bc = [
            load_cast("b1", b1d, [1, F], F32, F32),
            load_cast("b2", b2d, [1, F], F32, F32),
        ]
        Wl_sb = load_cast("Wl", Wld, [F, NCLS], F32, F32)
        bl_sb = load_cast("bl", bld, [1, NCLS], F32, F32)
        bl0_sb = load_cast("bl0", bl0d, [P, NB], F32, F32)
        bl1_sb = load_cast("bl1", bl1d, [P, NB], F32, F32)
        cnt_sb = load_cast("cnt", cntd, [P, NGT], F32, F32)
        gix_sb = load_cast("gix", gixd, [P, 2], I32, I32)
        eix_sb = load_cast("eix", eixd, [P, necol], I16, I16)

        # dinv = 1/sqrt(deg+1); rdinv = sqrt(deg+1) (transposed for bias matmul)
        deg_sb = craw.tile([P, NB], F32, tag="deg_sb", name="deg_sb")
        nc.sync.dma_start(out=deg_sb[:], in_=degd[:])
        sq_sb = T([P, NB], F32, name="sq_sb")
        nc.scalar.sqrt(sq_sb[:], deg_sb[:])
        dinv = T([P, NB], F32, name="dinv")
        nc.vector.reciprocal(dinv[:], sq_sb[:])


        # zero dram_sums (pool scatter target) early
        zsb = craw.tile([P, 512], F32, tag="zsb", name="zsb")
        nc.vector.memset(zsb[:], 0.0)
        nrow = G + 8
        r = 0
        while r < nrow:
            take = min(512, ((nrow - r) // P) * P)
            pp = P
            if take == 0:
                take = nrow - r
                pp = take
            nc.sync.dma_start(
                out=dram_sums[r : r + take, :].rearrange("(c p) f -> p c f", p=pp),
                in_=zsb[:pp, : take * F // pp].rearrange("p (c f) -> p c f", f=F),
            )
            r += take

        # ---------------- embedding ----------------
        for s in range(ESC):
            t0 = s * EGB
            t1 = min(t0 + EGB, NB)
            nt = t1 - t0
            nidx = nt * P
            g_tiles = []
            for j in range(3):
                g = msg_pool.tile([P, EGB, F], BF16, tag="msg")
                eoff = (j * ESC + s) * ECOLS
                nc.gpsimd.dma_gather(
                    out_ap=g[:, :nt, :],
                    in_ap=tabs[j][:, :],
                    idxs_ap=eix_sb[:, eoff : eoff + nidx // 16],
                    num_idxs=nidx,
                    num_idxs_reg=nidx,
                    elem_size=F,
                    single_packet=False,
                    queue_num=j,
                )
                g_tiles.append(g)
            nc.vector.tensor_tensor(
                out=g_tiles[0][:, :nt, :],
                in0=g_tiles[0][:, :nt, :],
                in1=g_tiles[1][:, :nt, :],
                op=mybir.AluOpType.add,
            )
            nc.vector.tensor_tensor(
                out=hA[:, t0 * F : t1 * F].rearrange("p (c f) -> p c f", f=F),
                in0=g_tiles[0][:, :nt, :],
                in1=g_tiles[2][:, :nt, :],
                op=mybir.AluOpType.add,
            )

        # ---------------- two GCN convs ----------------
        do_pool = PHASES >= 9
        pacc = [
            pacc_pool.tile([P, P], F32, tag=f"pacc{h}", name=f"pacc{h}")
            for h in range(2)
        ]
        for conv in range(min(2, max(0, PHASES - 1))):
            hin = hA if conv == 0 else hB
            hout = hB if conv == 0 else hA

            # b_bcast[n, f] = b[f] replicated down partitions (rank-1 via PE)
            bb_ps = xw_pool.tile([P, P], F32, tag="xw")
            nc.tensor.matmul(bb_ps[:], lhsT=ones_row[:], rhs=bc[conv][:],
                             start=True, stop=True)
            b_bcast = craw.tile([P, P], F32, tag=f"b_bcast{conv}", name=f"b_bcast{conv}")
            nc.scalar.copy(b_bcast[:], bb_ps[:])

            # xw phase: y = dinv * (h @ W) -> y_nm (SBUF) and y_slice (DRAM)
            for t in range(NB):
                tp = tp_pool.tile([P, P], BF16, tag="tp")
                nc.tensor.transpose(tp[:], hin[:, t * F : (t + 1) * F], id_bf[:])
                hT = sb_pool.tile([P, P], BF16, tag="hT")
                nc.scalar.copy(hT[:], tp[:])
                xw = xw_pool.tile([P, P], F32, tag="xw")
                nc.tensor.matmul(xw[:], lhsT=hT[:], rhs=Wc[conv][:], start=True, stop=True)
                nc.scalar.activation(
                    y_nm[:, t * F : (t + 1) * F],
                    xw[:],
                    mybir.ActivationFunctionType.Copy,
                    scale=dinv[:, t : t + 1],
                )
            # y_nm -> y_slice DRAM (node-major rows)
            full_b = NPC // P
            nc.sync.dma_start(
                out=y_slice[conv][: full_b * P, :].rearrange("(c p) f -> p c f", p=P),
                in_=y_nm[:, : full_b * F].rearrange("p (c f) -> p c f", f=F),
            )
            rem = NPC - full_b * P
            if rem:
                nc.sync.dma_start(
                    out=y_slice[conv][full_b * P :, :],
                    in_=y_nm[:rem, full_b * F : (full_b + 1) * F],
                )
            if NOAG:
                for kk in range(NCORES):
                    nc.sync.dma_start(
                        out=y_full[conv][kk * NPC : (kk + 1) * NPC, :],
                        in_=y_slice[conv][:],
                    )
            else:
                nc.gpsimd.collective_compute(
                    "AllGather",
                    mybir.AluOpType.bypass,
                    replica_groups=RG,
                    ins=[y_slice[conv][:]],
                    outs=[y_full[conv][:]],
                )

            # scatter phase
            ch_off = 0  # chunk offset into dst_cols
            col_off = 0  # column offset into edge_idx
            for blocks in (groups if SUB >= 2 else []):
                nblk = len(blocks)
                nch_q = nblk * Cq
                nidx = nch_q * P
                msgs = []
                for q in range(4):
                    ixt = ix_pool.tile([P, GB * Cq * 8], I16, tag="ix")
                    nc.sync.dma_start(
                        out=ixt[:, : nidx // 16],
                        in_=xixd[:, col_off : col_off + nidx // 16],
                    )
                    col_off += nidx // 16
                    msg = msg_pool.tile([P, GB * Cq, F], BF16, tag="msg")
                    nc.gpsimd.dma_gather(
                        out_ap=msg[:, :nch_q, :],
                        in_ap=y_full[conv][q * QROWS : (q + 1) * QROWS, :],
                        idxs_ap=ixt[:, : nidx // 16],
                        num_idxs=nidx,
                        num_idxs_reg=nidx,
                        elem_size=F,
                        single_packet=False,
                        queue_num=q,
                    )
                    msgs.append(msg)
                if SUB < 3:
                    ch_off += 4 * nblk * Cq
                    continue
                for bi, b in enumerate(blocks):
                    oh = oh_pool.tile([P, 4 * Cq, P], BF16, tag="oh")
                    nc.vector.tensor_tensor(
                        out=oh[:],
                        in0=iota_bf[:].unsqueeze(1).broadcast_to([P, 4 * Cq, P]),
                        in1=dstc_sb[:, ch_off + bi * 4 * Cq : ch_off + (bi + 1) * 4 * Cq]
                        .unsqueeze(2)
                        .broadcast_to([P, 4 * Cq, P]),
                        op=mybir.AluOpType.is_equal,
                    )
                    if SUB < 4:
                        continue
                    acc = acc_pool.tile([P, P], F32, tag="acc")
                    j = 0
                    for q in range(4):
                        for cc in range(Cq):
                            nc.tensor.matmul(
                                acc[:],
                                lhsT=oh[:, q * Cq + cc, :],
                                rhs=msgs[q][:, bi * Cq + cc, :],
                                start=(j == 0),
                                stop=(j == 4 * Cq - 1),
                            )
                            j += 1
                    # bias: bb = b (x) rdinv (cancels the later *dinv); ACT op
                    bb = sb_pool.tile([P, P], F32, tag="bb")
                    nc.scalar.activation(
                        bb[:], b_bcast[:], mybir.ActivationFunctionType.Copy,
                        scale=sq_sb[:, b : b + 1],
                    )
                    hs = sb_pool.tile([P, P], F32, tag="ep")
                    nc.vector.tensor_tensor(
                        out=hs[:],
                        in0=acc[:],
                        in1=y_nm[:, b * F : (b + 1) * F],
                        op=mybir.AluOpType.add,
                    )
                    nc.vector.tensor_tensor(
                        out=hs[:], in0=hs[:], in1=bb[:], op=mybir.AluOpType.add,
                    )
                    nc.vector.tensor_tensor(
                        out=hs[:],
                        in0=hs[:],
                        in1=dinv[:, b : b + 1].to_broadcast([P, P]),
                        op=mybir.AluOpType.mult,
                    )
                    nc.scalar.activation(
                        hout[:, b * F : (b + 1) * F],
                        hs[:],
                        mybir.ActivationFunctionType.Relu,
                    )
                    # mean-pool accumulation interleaved into conv2's epilogue
                    if conv == 1 and do_pool:
                        for hh in range(2):
                            bls = bl0_sb if hh == 0 else bl1_sb
                            oht = sb_pool.tile([P, P], BF16, tag="pooloh")
                            nc.vector.tensor_tensor(
                                out=oht[:],
                                in0=iota_f[:],
                                in1=bls[:, b : b + 1].to_broadcast([P, P]),
                                op=mybir.AluOpType.is_equal,
                            )
                            nc.tensor.matmul(
                                pacc[hh][:],
                                lhsT=oht[:],
                                rhs=hA[:, b * F : (b + 1) * F],
                                start=(b == 0),
                                stop=(b == NB - 1),
                            )
                ch_off += 4 * nch_q

        # ---------------- global mean pool + linear ----------------
        if do_pool:
            for h in range(2):
                se = sb_pool.tile([P, P], F32, tag="ep")
                nc.vector.tensor_copy(se[:], pacc[h][:])
                nc.gpsimd.indirect_dma_start(
                    out=dram_sums[:],
                    out_offset=bass.IndirectOffsetOnAxis(ap=gix_sb[:, h : h + 1], axis=0),
                    in_=se[:],
                    in_offset=None,
                )
            nc.gpsimd.collective_compute(
                "AllReduce",
                mybir.AluOpType.add,
                replica_groups=RG,
                ins=[dram_sums[:]],
                outs=[ar_sums[:]],
            )
            recip = T([P, NGT], F32, name="recip")
            nc.vector.reciprocal(recip[:], cnt_sb[:])
            for t in range(NGT):
                art = sb_pool.tile([P, P], F32, tag="art")
                nc.sync.dma_start(out=art[:], in_=ar_sums[t * P : (t + 1) * P, :])
                pooled = sb_pool.tile([P, P], F32, tag="ep")
                nc.vector.tensor_tensor(
                    out=pooled[:],
                    in0=art[:],
                    in1=recip[:, t : t + 1].to_broadcast([P, P]),
                    op=mybir.AluOpType.mult,
                )
                ptp = tp_pool.tile([P, P], F32, tag="tp")
                nc.tensor.transpose(ptp[:], pooled[:], id_f32[:])
                pooledT = sb_pool.tile([P, P], F32, tag="hT")
                nc.scalar.copy(pooledT[:], ptp[:])
                op = xw_pool.tile([P, NCLS], F32, tag="xw")
                nc.tensor.matmul(op[:], lhsT=pooledT[:], rhs=Wl_sb[:], start=True, stop=False)
                nc.tensor.matmul(op[:], lhsT=ones_row[:], rhs=bl_sb[:], start=False, stop=True)
                oute = sb_pool.tile([P, NCLS], F32, tag="oute")
                nc.vector.tensor_copy(oute[:], op[:])
                nc.sync.dma_start(out=outd[t * P : (t + 1) * P, :], in_=oute[:])


        else:
            dummy = sb_pool.tile([P, NCLS], F32, tag="oute", name="dummy")
            nc.vector.memset(dummy[:], 0.0)
            for t in range(NGT):
                nc.sync.dma_start(out=outd[t * P : (t + 1) * P, :], in_=dummy[:])

        ctx.close()
    nc.compile()
    return nc


_CACHE = {}


def _get_nc(cfg, Cq, pl, totcol, nch, necol):
    key = (tuple(sorted(cfg.items())), Cq, totcol, nch, necol)
    if key not in _CACHE:
        _CACHE[key] = _build(cfg, Cq, pl, totcol, nch, necol)
    return _CACHE[key]


def run(inputs, cfg, trace=False):
    x = np.asarray(inputs["x"])
    per_core, Cq, pl = _prep_host(x, np.asarray(inputs["edge_index"]),
                                  np.asarray(inputs["batch"]), cfg)
    totcol = per_core[0]["edge_idx"].shape[1]
    nch = per_core[0]["dst_cols"].shape[1]
    necol = per_core[0]["emb_idx"].shape[1]
    nc = _get_nc(cfg, Cq, pl, totcol, nch, necol)

    shared = dict(
        shape_tab=np.asarray(inputs["shape_tab"], np.float32).astype(NP_BF16),
        color_tab=np.asarray(inputs["color_tab"], np.float32).astype(NP_BF16),
        pos_tab=np.asarray(inputs["pos_tab"], np.float32).astype(NP_BF16),
        W1=np.asarray(inputs["W1"], np.float32),
        W2=np.asarray(inputs["W2"], np.float32),
        b1=np.asarray(inputs["b1"], np.float32).reshape(1, -1),
        b2=np.asarray(inputs["b2"], np.float32).reshape(1, -1),
        Wlin=np.asarray(inputs["Wlin"], np.float32),
        blin=np.asarray(inputs["blin"], np.float32).reshape(1, -1),
    )
    in_maps = [{**shared, **per_core[k]} for k in range(NCORES)]
    res = run_bass_kernel_spmd(nc, in_maps, list(range(NCORES)), trace=trace)
    out = np.asarray(res.results[0]["out"], np.float32)
    return out, res


def kernel(**inputs) -> np.ndarray:
    out, _ = run(inputs, CFG)
    return out



# revision 12
# speedup vs baseline: 1.8278x; 1.1474x over previous
"""GCN classifier (2x GCNConv + mean-pool + linear) on 8 Trainium2 NeuronCores.

Sharding: nodes (and their incident edges, partitioned by edge dst) are sharded
across the 8 cores; the small 128x128 weights are replicated; boundary node
features are exchanged with an AllGather of the scaled feature table after the
xw stage of each conv, before the per-edge gather/scatter.

All per-core differences are pushed into input *data* (the NEFF is SPMD: one
program for all 8 cores):
  - each core's edges are grouped into (dst-block of 128 nodes) x (src quadrant)
    cells, padded to a fixed number of 128-edge chunks (Cq) so the instruction
    stream is identical on every core
  - messages y[src] are fetched with dma_gather (int16 indices local to a src
    quadrant of 25000 rows), and scattered into PSUM with a one-hot(dst) matmul
  - degree counts / graph-id metadata are integer preprocessing done on host
"""

import math
import sys

sys.path.insert(0, "/opt/trn_rl_repo")

import ml_dtypes
import numpy as np

import concourse.bass as bass
import concourse.mybir as mybir
import concourse.tile as tile
from concourse import bacc
from concourse.bass_utils import run_bass_kernel_spmd
from concourse.masks import make_identity

BF16 = mybir.dt.bfloat16
F32 = mybir.dt.float32
I16 = mybir.dt.int16
I32 = mybir.dt.int32
NP_BF16 = ml_dtypes.bfloat16

P = 128
NCORES = 8

# problem sizes (hardcoded per the harness contract)
CFG = dict(N=100_000, E=1_600_000, G=1024, F=128, NCLS=10)

GB = 6  # dst blocks per gather group
EGB = 14  # node tiles per embedding-gather superchunk


def _plan(cfg):
    N, G = cfg["N"], cfg["G"]
    p = {}
    p["NPC"] = N // NCORES  # nodes per core
    p["NB"] = math.ceil(p["NPC"] / P)  # 128-node blocks per core
    p["NBP"] = p["NB"] * P
    p["QROWS"] = N // 4  # src quadrant rows (must be < 32768)
    assert p["QROWS"] < 32768
    p["groups"] = [
        list(range(g, min(g + GB, p["NB"]))) for g in range(0, p["NB"], GB)
    ]
    p["ESC"] = math.ceil(p["NB"] / EGB)  # embed superchunks
    p["ECOLS"] = EGB * P // 16  # idx cols per embed call
    p["NGT"] = G // P  # graph tiles
    assert G % P == 0
    return p


def _wrap16(idx_flat):
    """int16 index list -> [128, n/16] wrapped in 16 partitions, replicated 8x."""
    return np.tile(idx_flat.reshape(-1, 16).T, (8, 1))


def _prep_host(x, edge_index, batch, cfg):
    """Integer/index preprocessing + per-core metadata. Returns (per_core, Cq)."""
    pl = _plan(cfg)
    N, G = cfg["N"], cfg["G"]
    NPC, NB, NBP, QROWS = pl["NPC"], pl["NB"], pl["NBP"], pl["QROWS"]

    src = np.asarray(edge_index[0], np.int64)
    dst = np.asarray(edge_index[1], np.int64)
    batch = np.asarray(batch, np.int64)
    x = np.asarray(x, np.int64)

    deg_p1 = (np.bincount(dst, minlength=N) + 1).astype(np.float32)
    cnt = np.maximum(np.bincount(batch, minlength=G), 1).astype(np.float32)
    cnt_pt = cnt.reshape(pl["NGT"], P).T.copy()  # [P, NGT]

    core_of = dst // NPC
    per_core_edge = []  # (sorted sloc, sorted w, counts per cell)
    Cq = 1
    for k in range(NCORES):
        m = core_of == k
        s_k, d_k = src[m], dst[m] - k * NPC
        blk = d_k >> 7
        q = s_k // QROWS
        key = (blk * 4 + q).astype(np.int64)
        order = np.argsort(key, kind="stable")
        counts = np.bincount(key, minlength=NB * 4)
        Cq = max(Cq, math.ceil(counts.max() / P))
        sloc = (s_k - q * QROWS).astype(np.int16)[order]
        w = (d_k & 127).astype(np.float32)[order]
        per_core_edge.append((sloc, w, counts))

    per_core = []
    for k in range(NCORES):
        sloc, w, counts = per_core_edge[k]
        cap = Cq * P
        src_pad = np.zeros((NB * 4, cap), np.int16)
        dst_pad = np.full((NB * 4, cap), -1.0, np.float32)
        starts = np.concatenate([[0], np.cumsum(counts)])
        for cell in range(NB * 4):
            c0, c1 = starts[cell], starts[cell + 1]
            n = c1 - c0
            if n:
                src_pad[cell, :n] = sloc[c0:c1]
                dst_pad[cell, :n] = w[c0:c1]

        idx_cols, dst_cols = [], []
        for blocks in pl["groups"]:
            for q in range(4):
                cells = [b * 4 + q for b in blocks]
                flat = src_pad[cells].reshape(-1)
                idx_cols.append(_wrap16(flat))
            for b in blocks:
                # block-major: the 4*Cq chunk columns of block b, (q, cc) order
                cells = [b * 4 + q for q in range(4)]
                dst_cols.append(dst_pad[cells].reshape(-1, P).T)
        edge_idx = np.concatenate(idx_cols, 1)  # [128, TOTCOL] i16
        dstc = np.concatenate(dst_cols, 1).astype(NP_BF16)  # [128, NCH]

        # degree (layout [p, c] = local node c*128+p), pad nodes -> deg+1 = 1
        dp = np.ones(NBP, np.float32)
        dp[:NPC] = deg_p1[k * NPC : (k + 1) * NPC]
        dp = dp.reshape(NB, P).T.copy()

        # pool metadata
        bl = batch[k * NPC : (k + 1) * NPC]
        gbase = int(bl[0])
        gspan = int(bl[-1]) - gbase + 1
        assert gspan <= 2 * P, f"core {k} graph span {gspan} > 256"
        blf = np.full(NBP, -1.0, np.float32)
        blf[:NPC] = (bl - gbase).astype(np.float32)
        bl0 = blf.reshape(NB, P).T.astype(np.float32)
        bl1 = (blf - P).reshape(NB, P).T.astype(np.float32)
        gidx = np.zeros((P, 2), np.int32)
        for h in range(2):
            v = gbase + h * P + np.arange(P)
            v = np.where(v < G, v, G + (v % 8))
            gidx[:, h] = v

        # embedding gather indices (x values < 256 fit int16)
        xi = np.zeros((NBP, 3), np.int16)
        xi[:NPC] = x[k * NPC : (k + 1) * NPC].astype(np.int16)
        ecols = []
        for j in range(3):
            for s in range(pl["ESC"]):
                seg = np.zeros(EGB * P, np.int16)
                src_seg = xi[s * EGB * P : (s + 1) * EGB * P, j]
                seg[: len(src_seg)] = src_seg
                ecols.append(_wrap16(seg))
        emb_idx = np.concatenate(ecols, 1)

        per_core.append(
            dict(
                deg_p1=dp,
                bl0=bl0,
                bl1=bl1,
                gidx=gidx,
                cnt=cnt_pt,
                emb_idx=emb_idx,
                edge_idx=edge_idx,
                dst_cols=dstc,
            )
        )
    return per_core, Cq, pl


def _build(cfg, Cq, pl, totcol, nch, necol):
    """Build the SPMD Bass program (one NEFF for all 8 cores)."""
    import os
    PHASES = int(os.environ.get("K_PHASES", "9"))  # 1=embed 2=+conv1 3=+conv2 9=all
    SUB = int(os.environ.get("K_SUB", "9"))  # 1=xw+AG 2=+gathers 3=+onehot 4=+matmul/epi
    NOAG = int(os.environ.get("K_NOAG", "0"))  # 1: replace AllGather with local copies
    N, G, F, NCLS = cfg["N"], cfg["G"], cfg["F"], cfg["NCLS"]
    NPC, NB, QROWS, NGT = pl["NPC"], pl["NB"], pl["QROWS"], pl["NGT"]
    groups, ESC, ECOLS = pl["groups"], pl["ESC"], pl["ECOLS"]

    nc = bacc.Bacc("TRN2", num_devices=NCORES, num_swdge_queues=4)
    RG = [list(range(NCORES))]

    # ---- I/O ----
    tabs = [
        nc.dram_tensor("shape_tab", [16, F], BF16, kind="ExternalInput"),
        nc.dram_tensor("color_tab", [16, F], BF16, kind="ExternalInput"),
        nc.dram_tensor("pos_tab", [256, F], BF16, kind="ExternalInput"),
    ]
    W1d = nc.dram_tensor("W1", [F, F], F32, kind="ExternalInput")
    W2d = nc.dram_tensor("W2", [F, F], F32, kind="ExternalInput")
    b1d = nc.dram_tensor("b1", [1, F], F32, kind="ExternalInput")
    b2d = nc.dram_tensor("b2", [1, F], F32, kind="ExternalInput")
    Wld = nc.dram_tensor("Wlin", [F, NCLS], F32, kind="ExternalInput")
    bld = nc.dram_tensor("blin", [1, NCLS], F32, kind="ExternalInput")
    degd = nc.dram_tensor("deg_p1", [P, NB], F32, kind="ExternalInput")
    bl0d = nc.dram_tensor("bl0", [P, NB], F32, kind="ExternalInput")
    bl1d = nc.dram_tensor("bl1", [P, NB], F32, kind="ExternalInput")
    gixd = nc.dram_tensor("gidx", [P, 2], I32, kind="ExternalInput")
    cntd = nc.dram_tensor("cnt", [P, NGT], F32, kind="ExternalInput")
    eixd = nc.dram_tensor("emb_idx", [P, necol], I16, kind="ExternalInput")
    xixd = nc.dram_tensor("edge_idx", [P, totcol], I16, kind="ExternalInput")
    dcd = nc.dram_tensor("dst_cols", [P, nch], BF16, kind="ExternalInput")
    outd = nc.dram_tensor("out", [G, NCLS], F32, kind="ExternalOutput")

    with tile.TileContext(nc) as tc:
        import contextlib

        ctx = contextlib.ExitStack()
        persist = ctx.enter_context(tc.tile_pool(name="persist", bufs=1))
        dramp = ctx.enter_context(tc.tile_pool(name="dramp", bufs=1, space="DRAM"))
        tp_pool = ctx.enter_context(tc.tile_pool(name="tp", bufs=2, space="PSUM"))
        xw_pool = ctx.enter_context(tc.tile_pool(name="xw", bufs=2, space="PSUM"))
        acc_pool = ctx.enter_context(tc.tile_pool(name="acc", bufs=2, space="PSUM"))
        pacc_pool = ctx.enter_context(tc.tile_pool(name="pacc", bufs=1, space="PSUM"))
        sb_pool = ctx.enter_context(tc.tile_pool(name="work", bufs=3))
        msg_pool = ctx.enter_context(tc.tile_pool(name="msg", bufs=12))
        oh_pool = ctx.enter_context(tc.tile_pool(name="oh", bufs=3))
        ix_pool = ctx.enter_context(tc.tile_pool(name="ix", bufs=12))
        craw = ctx.enter_context(tc.tile_pool(name="craw", bufs=1))

        def T(shape, dt, space=None, addr_space="Local", name=None):
            pool = dramp if space == "DRAM" else persist
            return pool.tile(shape, dt, tag=name, name=name, addr_space=addr_space)

        # ---- internal DRAM ----
        y_slice = [
            T([NPC, F], BF16, space="DRAM", name=f"y_slice{c}") for c in range(2)
        ]
        y_full = [
            T([NCORES * NPC, F], BF16, space="DRAM",
              addr_space="Local" if NOAG else "Shared", name=f"y_full{c}")
            for c in range(2)
        ]
        dram_sums = T([G + 8, F], F32, space="DRAM", name="dram_sums")
        ar_sums = T([G + 8, F], F32, space="DRAM", addr_space="Shared",
                          name="ar_sums")

        # ---- persistent SBUF ----
        hA = T([P, NB * F], BF16, name="hA")
        hB = hA  # in-place: h(in) is dead once y_nm is computed in the xw phase
        y_nm = T([P, NB * F], BF16, name="y_nm")
        dstc_sb = T([P, nch], BF16, name="dstc_sb")
        nc.sync.dma_start(out=dstc_sb[:], in_=dcd[:])

        # constants
        iota_i = craw.tile([P, P], I32, tag="iota_i", name="iota_i")
        nc.gpsimd.iota(iota_i[:], pattern=[[1, P]], base=0, channel_multiplier=0)
        iota_bf = T([P, P], BF16, name="iota_bf")
        nc.vector.tensor_copy(iota_bf[:], iota_i[:])
        iota_f = T([P, P], F32, name="iota_f")
        nc.vector.tensor_copy(iota_f[:], iota_i[:])
        id_f32 = T([P, P], F32, name="id_f32")
        make_identity(nc, id_f32[:])
        id_bf = T([P, P], BF16, name="id_bf")
        nc.vector.tensor_copy(id_bf[:], id_f32[:])
        ones_row = T([1, P], F32, name="ones_row")
        nc.vector.memset(ones_row[:], 1.0)

        def load_cast(name, dram, shape, dt_in, dt_out):
            t = T(shape, dt_out, name=name)
            if dt_out == dt_in:
                nc.sync.dma_start(out=t[:], in_=dram[:])
            else:
                # NB: SWDGE cast-DMA + indirect_dma in one program crashes the
                # device (observed NRT_EXEC_UNIT_UNRECOVERABLE) - cast on DVE.
                raw = craw.tile(shape, dt_in, tag=name + "_r", name=name + "_r")
                nc.sync.dma_start(out=raw[:], in_=dram[:])
                nc.vector.tensor_copy(t[:], raw[:])
            return t

        Wc = [
            load_cast("W1", W1d, [F, F], F32, BF16),
            load_cast("W2", W2d, [F, F], F32, BF16),
        ]
        bc = [
            load_cast("b1", b1d, [1, F], F32, F32),
            load_cast("b2", b2d, [1, F], F32, F32),
        ]
        Wl_sb = load_cast("Wl", Wld, [F, NCLS], F32, F32)
        bl_sb = load_cast("bl", bld, [1, NCLS], F32, F32)
        bl0_sb = load_cast("bl0", bl0d, [P, NB], F32, F32)
        bl1_sb = load_cast("bl1", bl1d, [P, NB], F32, F32)
        cnt_sb = load_cast("cnt", cntd, [P, NGT], F32, F32)
        gix_sb = load_cast("gix", gixd, [P, 2], I32, I32)
        eix_sb = load_cast("eix", eixd, [P, necol], I16, I16)

        # dinv = 1/sqrt(deg+1); rdinv = sqrt(deg+1) (transposed for bias matmul)
        deg_sb = craw.tile([P, NB], F32, tag="deg_sb", name="deg_sb")
        nc.sync.dma_start(out=deg_sb[:], in_=degd[:])
        sq_sb = T([P, NB], F32, name="sq_sb")
        nc.scalar.sqrt(sq_sb[:], deg_sb[:])
        dinv = T([P, NB], F32, name="dinv")
        nc.vector.reciprocal(dinv[:], sq_sb[:])


        # zero dram_sums (pool scatter target) early
        zsb = craw.tile([P, 512], F32, tag="zsb", name="zsb")
        nc.vector.memset(zsb[:], 0.0)
        nrow = G + 8
        r = 0
        while r < nrow:
            take = min(512, ((nrow - r) // P) * P)
            pp = P
            if take == 0:
                take = nrow - r
                pp = take
            nc.sync.dma_start(
                out=dram_sums[r : r + take, :].rearrange("(c p) f -> p c f", p=pp),
                in_=zsb[:pp, : take * F // pp].rearrange("p (c f) -> p c f", f=F),
            )
            r += take

        # ---------------- embedding ----------------
        for s in range(ESC):
            t0 = s * EGB
            t1 = min(t0 + EGB, NB)
            nt = t1 - t0
            nidx = nt * P
            g_tiles = []
            for j in range(3):
                g = msg_pool.tile([P, EGB, F], BF16, tag="msg")
                eoff = (j * ESC + s) * ECOLS
                nc.gpsimd.dma_gather(
                    out_ap=g[:, :nt, :],
                    in_ap=tabs[j][:, :],
                    idxs_ap=eix_sb[:, eoff : eoff + nidx // 16],
                    num_idxs=nidx,
                    num_idxs_reg=nidx,
                    elem_size=F,
                    single_packet=False,
                    queue_num=j,
                )
                g_tiles.append(g)
            nc.vector.tensor_tensor(
                out=g_tiles[0][:, :nt, :],
                in0=g_tiles[0][:, :nt, :],
                in1=g_tiles[1][:, :nt, :],
                op=mybir.AluOpType.add,
            )
            nc.vector.tensor_tensor(
                out=hA[:, t0 * F : t1 * F].rearrange("p (c f) -> p c f", f=F),
                in0=g_tiles[0][:, :nt, :],
                in1=g_tiles[2][:, :nt, :],
                op=mybir.AluOpType.add,
            )

        # ---------------- two GCN convs ----------------
        do_pool = PHASES >= 9
        pacc = [
            pacc_pool.tile([P, P], F32, tag=f"pacc{h}", name=f"pacc{h}")
            for h in range(2)
        ]
        for conv in range(min(2, max(0, PHASES - 1))):
            hin = hA if conv == 0 else hB
            hout = hB if conv == 0 else hA

            # b_bcast[n, f] = b[f] replicated down partitions (rank-1 via PE)
            bb_ps = xw_pool.tile([P, P], F32, tag="xw")
            nc.tensor.matmul(bb_ps[:], lhsT=ones_row[:], rhs=bc[conv][:],
                             start=True, stop=True)
            b_bcast = craw.tile([P, P], F32, tag=f"b_bcast{conv}", name=f"b_bcast{conv}")
            nc.scalar.copy(b_bcast[:], bb_ps[:])

            # xw phase: y = dinv * (h @ W) -> y_nm (SBUF) and y_slice (DRAM)
            for t in range(NB):
                tp = tp_pool.tile([P, P], BF16, tag="tp")
                nc.tensor.transpose(tp[:], hin[:, t * F : (t + 1) * F], id_bf[:])
                hT = sb_pool.tile([P, P], BF16, tag="hT")
                nc.scalar.copy(hT[:], tp[:])
                xw = xw_pool.tile([P, P], F32, tag="xw")
                nc.tensor.matmul(xw[:], lhsT=hT[:], rhs=Wc[conv][:], start=True, stop=True)
                nc.scalar.activation(
                    y_nm[:, t * F : (t + 1) * F],
                    xw[:],
                    mybir.ActivationFunctionType.Copy,
                    scale=dinv[:, t : t + 1],
                )
            # y_nm -> y_slice DRAM (node-major rows)
            full_b = NPC // P
            nc.sync.dma_start(
                out=y_slice[conv][: full_b * P, :].rearrange("(c p) f -> p c f", p=P),
                in_=y_nm[:, : full_b * F].rearrange("p (c f) -> p c f", f=F),
            )
            rem = NPC - full_b * P
            if rem:
                nc.sync.dma_start(
                    out=y_slice[conv][full_b * P :, :],
                    in_=y_nm[:rem, full_b * F : (full_b + 1) * F],
                )
            if NOAG:
                for kk in range(NCORES):
                    nc.sync.dma_start(
                        out=y_full[conv][kk * NPC : (kk + 1) * NPC, :],
                        in_=y_slice[conv][:],
                    )
            else:
                nc.gpsimd.collective_compute(
                    "AllGather",
                    mybir.AluOpType.bypass,
                    replica_groups=RG,
                    ins=[y_slice[conv][:]],
                    outs=[y_full[conv][:]],
                )

            # scatter phase
            ch_off = 0  # chunk offset into dst_cols
            col_off = 0  # column offset into edge_idx
            for blocks in (groups if SUB >= 2 else []):
                nblk = len(blocks)
                nch_q = nblk * Cq
                nidx = nch_q * P
                msgs = []
                for q in range(4):
                    ixt = ix_pool.tile([P, GB * Cq * 8], I16, tag="ix")
                    nc.sync.dma_start(
                        out=ixt[:, : nidx // 16],
                        in_=xixd[:, col_off : col_off + nidx // 16],
                    )
                    col_off += nidx // 16
                    msg = msg_pool.tile([P, GB * Cq, F], BF16, tag="msg")
                    nc.gpsimd.dma_gather(
                        out_ap=msg[:, :nch_q, :],
                        in_ap=y_full[conv][q * QROWS : (q + 1) * QROWS, :],
                        idxs_ap=ixt[:, : nidx // 16],
                        num_idxs=nidx,
                        num_idxs_reg=nidx,
                        elem_size=F,
                        single_packet=False,
                        queue_num=q,
                    )
                    msgs.append(msg)
                if SUB < 3:
                    ch_off += 4 * nblk * Cq
                    continue
                for bi, b in enumerate(blocks):
                    oh = oh_pool.tile([P, 4 * Cq, P], BF16, tag="oh")
                    nc.vector.tensor_tensor(
                        out=oh[:],
                        in0=iota_bf[:].unsqueeze(1).broadcast_to([P, 4 * Cq, P]),
                        in1=dstc_sb[:, ch_off + bi * 4 * Cq : ch_off + (bi + 1) * 4 * Cq]
                        .unsqueeze(2)
                        .broadcast_to([P, 4 * Cq, P]),
                        op=mybir.AluOpType.is_equal,
                    )
                    if SUB < 4:
                        continue
                    acc = acc_pool.tile([P, P], F32, tag="acc")
                    j = 0
                    for q in range(4):
                        for cc in range(Cq):
                            nc.tensor.matmul(
                                acc[:],
                                lhsT=oh[:, q * Cq + cc, :],
                                rhs=msgs[q][:, bi * Cq + cc, :],
                                start=(j == 0),
                                stop=(j == 4 * Cq - 1),
                            )
                            j += 1
                    # bias: bb = b (x) rdinv (cancels the later *dinv); ACT op
                    bb = sb_pool.tile([P, P], F32, tag="bb")
                    nc.scalar.activation(
                        bb[:], b_bcast[:], mybir.ActivationFunctionType.Copy,
                        scale=sq_sb[:, b : b + 1],
                    )
                    hs = sb_pool.tile([P, P], F32, tag="ep")
                    nc.vector.tensor_tensor(
                        out=hs[:],
                        in0=acc[:],
                        in1=y_nm[:, b * F : (b + 1) * F],
                        op=mybir.AluOpType.add,
                    )
                    nc.vector.tensor_tensor(
                        out=hs[:], in0=hs[:], in1=bb[:], op=mybir.AluOpType.add,
                    )
                    nc.vector.tensor_tensor(
                        out=hs[:],
                        in0=hs[:],
                        in1=dinv[:, b : b + 1].to_broadcast([P, P]),
                        op=mybir.AluOpType.mult,
                    )
                    nc.scalar.activation(
                        hout[:, b * F : (b + 1) * F],
                        hs[:],
                        mybir.ActivationFunctionType.Relu,
                    )
                    # mean-pool accumulation interleaved into conv2's epilogue
                    if conv == 1 and do_pool:
                        for hh in range(2):
                            bls = bl0_sb if hh == 0 else bl1_sb
                            oht = sb_pool.tile([P, P], BF16, tag="pooloh")
                            nc.vector.tensor_tensor(
                                out=oht[:],
                                in0=iota_f[:],
                                in1=bls[:, b : b + 1].to_broadcast([P, P]),
                                op=mybir.AluOpType.is_equal,
                            )
                            nc.tensor.matmul(
                                pacc[hh][:],
                                lhsT=oht[:],
                                rhs=hA[:, b * F : (b + 1) * F],
                                start=(b == 0),
                                stop=(b == NB - 1),
                            )
                ch_off += 4 * nch_q

        # ---------------- global mean pool + linear ----------------
        if do_pool:
            for h in range(2):
                se = sb_pool.tile([P, P], F32, tag="ep")
                nc.vector.tensor_copy(se[:], pacc[h][:])
                nc.gpsimd.indirect_dma_start(
                    out=dram_sums[:],
                    out_offset=bass.IndirectOffsetOnAxis(ap=gix_sb[:, h : h + 1], axis=0),
                    in_=se[:],
                    in_offset=None,
                )
            nc.gpsimd.collective_compute(
                "AllReduce",
                mybir.AluOpType.add,
                replica_groups=RG,
                ins=[dram_sums[:]],
                outs=[ar_sums[:]],
            )
            recip = T([P, NGT], F32, name="recip")
            nc.vector.reciprocal(recip[:], cnt_sb[:])
            for t in range(NGT):
                art = sb_pool.tile([P, P], F32, tag="art")
                nc.sync.dma_start(out=art[:], in_=ar_sums[t * P : (t + 1) * P, :])
                pooled = sb_pool.tile([P, P], F32, tag="ep")
                nc.vector.tensor_tensor(
                    out=pooled[:],
                    in0=art[:],
                    in1=recip[:, t : t + 1].to_broadcast([P, P]),
                    op=mybir.AluOpType.mult,
                )
                ptp = tp_pool.tile([P, P], F32, tag="tp")
                nc.tensor.transpose(ptp[:], pooled[:], id_f32[:])
                pooledT = sb_pool.tile([P, P], F32, tag="hT")
                nc.scalar.copy(pooledT[:], ptp[:])
                op = xw_pool.tile([P, NCLS], F32, tag="xw")
                nc.tensor.matmul(op[:], lhsT=pooledT[:], rhs=Wl_sb[:], start=True, stop=False)
                nc.tensor.matmul(op[:], lhsT=ones_row[:], rhs=bl_sb[:], start=False, stop=True)
                oute = sb_pool.tile([P, NCLS], F32, tag="oute")
                nc.vector.tensor_copy(oute[:], op[:])
                nc.sync.dma_start(out=outd[t * P : (t + 1) * P, :], in_=oute[:])


        else:
            dummy = sb_pool.tile([P, NCLS], F32, tag="oute", name="dummy")
            nc.vector.memset(dummy[:], 0.0)
            for t in range(NGT):
                nc.sync.dma_start(out=outd[t * P : (t + 1) * P, :], in_=dummy[:])

        ctx.close()
    nc.compile()
    return nc


_CACHE = {}


def _get_nc(cfg, Cq, pl, totcol, nch, necol):
    key = (tuple(sorted(cfg.items())), Cq, totcol, nch, necol)
    if key not in _CACHE:
        _CACHE[key] = _build(cfg, Cq, pl, totcol, nch, necol)
    return _CACHE[key]


def run(inputs, cfg, trace=False):
    x = np.asarray(inputs["x"])
    per_core, Cq, pl = _prep_host(x, np.asarray(inputs["edge_index"]),
                                  np.asarray(inputs["batch"]), cfg)
    totcol = per_core[0]["edge_idx"].shape[1]
    nch = per_core[0]["dst_cols"].shape[1]
    necol = per_core[0]["emb_idx"].shape[1]
    nc = _get_nc(cfg, Cq, pl, totcol, nch, necol)

    shared = dict(
        shape_tab=np.asarray(inputs["shape_tab"], np.float32).astype(NP_BF16),
        color_tab=np.asarray(inputs["color_tab"], np.float32).astype(NP_BF16),
        pos_tab=np.asarray(inputs["pos_tab"], np.float32).astype(NP_BF16),
        W1=np.asarray(inputs["W1"], np.float32),
        W2=np.asarray(inputs["W2"], np.float32),
        b1=np.asarray(inputs["b1"], np.float32).reshape(1, -1),
        b2=np.asarray(inputs["b2"], np.float32).reshape(1, -1),
        Wlin=np.asarray(inputs["Wlin"], np.float32),
        blin=np.asarray(inputs["blin"], np.float32).reshape(1, -1),
    )
    in_maps = [{**shared, **per_core[k]} for k in range(NCORES)]
    res = run_bass_kernel_spmd(nc, in_maps, list(range(NCORES)), trace=trace)
    out = np.asarray(res.results[0]["out"], np.float32)
    return out, res


def kernel(**inputs) -> np.ndarray:
    out, _ = run(inputs, CFG)
    return out

